# revision 1
# baseline (speedup 1.0000x reference)
"""Trainium2 Bass kernel for InpaintingAttnProcessor (3-branch masked SDPA).

Sharding: heads of the two 8-head SDPA branches are split across the 8
cores (1 head each); the single-head d=640 "entity" branch is sharded over
query rows (each core owns 512 queries and computes its k/v projections
locally). Masks are fused into the score matmul as extra contraction rows
(+/- 2^17 bias), softmax runs without max-subtraction (scores are O(5);
masked lanes underflow to exactly 0). One ReduceScatter combines the
per-head Wo partial products; everything else is local.
"""
import numpy as np
from contextlib import ExitStack

import concourse.bass as bass
import concourse.tile as tile
from concourse import bacc, mybir
from concourse.bass_utils import run_bass_kernel_spmd

S, C, H, D = 4096, 640, 8, 80
NCORES = 8
SL = S // NCORES          # 512 queries per core (ent branch + output slice)
BB = 131072.0             # mask bias magnitude (2^17, exact in bf16)
SCALE_H = 1.0 / np.sqrt(80.0)
SCALE_E = 1.0 / np.sqrt(640.0)
F32 = mybir.dt.float32
BF16 = mybir.dt.bfloat16
I32 = mybir.dt.int32
EXP = mybir.ActivationFunctionType.Exp
COPY = mybir.ActivationFunctionType.Copy
EQ = mybir.AluOpType.is_equal
MULT = mybir.AluOpType.mult
ADD = mybir.AluOpType.add

_cache = {}


def _build():
    nc = bacc.Bacc("TRN2", target_bir_lowering=False, debug=False,
                   num_devices=NCORES)
    d = {}
    d["hT"] = nc.dram_tensor("hT", [C, S], F32, kind="ExternalInput")
    d["hq"] = nc.dram_tensor("hq", [C, SL], F32, kind="ExternalInput")
    d["res"] = nc.dram_tensor("res", [SL, C], F32, kind="ExternalInput")
    for w in ("wq", "wk", "wv", "wqo", "wko", "wvo"):
        d[w] = nc.dram_tensor(w, [C, D], F32, kind="ExternalInput")
    for w in ("wqe", "wke", "wve", "wof"):
        d[w] = nc.dram_tensor(w, [C, C], F32, kind="ExternalInput")
    d["woh"] = nc.dram_tensor("woh", [D, C], F32, kind="ExternalInput")
    d["mrow"] = nc.dram_tensor("mrow", [1, S], I32, kind="ExternalInput")
    d["imrow"] = nc.dram_tensor("imrow", [1, S], I32, kind="ExternalInput")
    d["mq"] = nc.dram_tensor("mq", [1, SL], I32, kind="ExternalInput")
    out_d = nc.dram_tensor("out", [SL, C], F32, kind="ExternalOutput")
    P_dram = nc.dram_tensor("P_part", [S, C], F32)
    Pred_dram = nc.dram_tensor("P_red", [SL, C], F32)

    with tile.TileContext(nc) as tc:
        _body(nc, tc, d, out_d, P_dram, Pred_dram)
    nc.compile()
    return nc


def _body(nc, tc, d, out_d, P_dram, Pred_dram):
    ctx = ExitStack()
    with ctx:
        base = ctx.enter_context(tc.tile_pool(name="base", bufs=1))

        # ---------- load + bf16-convert inputs ----------
        hTb = base.tile([128, 5 * S], BF16, tag="hTb")
        wsb = {}
        for w in ("wq", "wk", "wv", "wqo", "wko", "wvo"):
            wsb[w] = base.tile([128, 5 * D], BF16, tag="w_" + w, name="wsb_" + w)
        woh_sb = base.tile([D, C], BF16, tag="woh")

        with tc.tile_pool(name="stage", bufs=2) as stage:
            for cc in range(5):
                st_t = stage.tile([128, S], F32, tag="stg")
                nc.sync.dma_start(st_t[:], d["hT"].ap()[cc * 128:(cc + 1) * 128, :])
                nc.vector.tensor_copy(hTb[:, cc * S:(cc + 1) * S], st_t[:])
            for w in ("wq", "wk", "wv", "wqo", "wko", "wvo"):
                st_t = stage.tile([128, S], F32, tag="stg")
                for cc in range(5):
                    nc.sync.dma_start(st_t[:, cc * D:(cc + 1) * D],
                                      d[w].ap()[cc * 128:(cc + 1) * 128, :])
                nc.vector.tensor_copy(wsb[w][:], st_t[:, 0:5 * D])
            st_t = stage.tile([128, S], F32, tag="stg")
            nc.sync.dma_start(st_t[0:D, 0:C], d["woh"].ap()[:])
            nc.vector.tensor_copy(woh_sb[:], st_t[0:D, 0:C])

        # ---------- mask-derived bias rows ----------
        augk = base.tile([5, S], BF16, tag="augk")    # [onehot_k; 1]
        augko = base.tile([5, S], BF16, tag="augko")  # [onehot_k*im0_k; 1]
        augq = base.tile([5, S], BF16, tag="augq")    # [B*onehot_q; -B]
        augqe = base.tile([5, SL], BF16, tag="augqe")  # ent q-slice bias rows
        iot4i = base.tile([4, 1], I32, tag="iot4i")
        nc.gpsimd.iota(iot4i[:], [[0, 1]], channel_multiplier=1)
        iot4 = base.tile([4, 1], F32, tag="iot4")
        nc.vector.tensor_copy(iot4[:], iot4i[:])
        with tc.tile_pool(name="maskp", bufs=3) as mp:
            mi = mp.tile([4, S], I32, tag="tmp", name="mi")
            for p in range(4):
                nc.sync.dma_start(mi[p:p + 1, :], d["mrow"].ap()[0:1, :])
            mf = mp.tile([4, S], F32, tag="tmp", name="mf")
            nc.vector.tensor_copy(mf[:], mi[:])
            oh = mp.tile([4, S], F32, tag="oh", name="oh")
            nc.vector.tensor_scalar(oh[:], mf[:], iot4[:], None, op0=EQ)
            imi = mp.tile([4, S], I32, tag="tmp", name="imi")
            for p in range(4):
                nc.sync.dma_start(imi[p:p + 1, :], d["imrow"].ap()[0:1, :])
            imf = mp.tile([4, S], F32, tag="tmp", name="imf")
            nc.vector.tensor_copy(imf[:], imi[:])
            im0 = mp.tile([4, S], F32, tag="tmp", name="im0")
            nc.vector.tensor_scalar(im0[:], imf[:], 0.0, None, op0=EQ)
            nc.vector.memset(augk[:], 1.0)
            nc.vector.tensor_copy(augk[0:4, :], oh[:])
            oh0 = mp.tile([4, S], F32, tag="tmp", name="oh0")
            nc.vector.tensor_mul(oh0[:], oh[:], im0[:])
            nc.vector.memset(augko[:], 1.0)
            nc.vector.tensor_copy(augko[0:4, :], oh0[:])
            nc.vector.memset(augq[:], -BB)
            nc.vector.tensor_scalar(augq[0:4, :], oh[:], BB, None, op0=MULT)
            mqi = mp.tile([4, SL], I32, tag="tmp", name="mqi")
            for p in range(4):
                nc.sync.dma_start(mqi[p:p + 1, :], d["mq"].ap()[0:1, :])
            mqf = mp.tile([4, SL], F32, tag="tmp", name="mqf")
            nc.vector.tensor_copy(mqf[:], mqi[:])
            nc.vector.memset(augqe[:], -BB)
            nc.vector.tensor_scalar(augqe[0:4, :], mqf[:], iot4[:], BB,
                                    op0=EQ, op1=MULT)

        ones_bf = base.tile([128, 1], BF16, tag="ones_bf")
        nc.vector.memset(ones_bf[:], 1.0)
        ones_f = base.tile([1, 128], F32, tag="ones_f")
        nc.vector.memset(ones_f[:], 1.0)

        entout = base.tile([128, 4 * C], F32, tag="entout")
        hsTo = base.tile([D, S], BF16, tag="hsTo")
        hsTu = base.tile([D, S], BF16, tag="hsTu")

        # ================= ENT branch (local, q-slice) =================
        # qeT projection [640, SL] bf16, cc-major over d-chunks
        with tc.tile_pool(name="entp", bufs=1) as ep:
            for w in ("wqe", "wke", "wve", "wof"):
                wsb[w] = ep.tile([128, 5 * C], BF16, tag="w_" + w,
                                 name="wsb_" + w)
            hqb = ep.tile([128, 5 * SL], BF16, tag="hqb")
            with tc.tile_pool(name="stage2", bufs=2) as stage2:
                for w in ("wqe", "wke", "wve", "wof"):
                    for cc in range(5):
                        st2 = stage2.tile([128, C], F32, tag="stg2", name="st2")
                        nc.sync.dma_start(st2[:],
                                          d[w].ap()[cc * 128:(cc + 1) * 128, :])
                        nc.vector.tensor_copy(wsb[w][:, cc * C:(cc + 1) * C],
                                              st2[:])
                for cc in range(5):
                    st2 = stage2.tile([128, C], F32, tag="stg2", name="st2")
                    nc.sync.dma_start(st2[:, 0:SL],
                                      d["hq"].ap()[cc * 128:(cc + 1) * 128, :])
                    nc.vector.tensor_copy(hqb[:, cc * SL:(cc + 1) * SL],
                                          st2[:, 0:SL])
            qeb = ep.tile([128, 5 * SL], BF16, tag="qeb")
            eph = ctx_ent = ExitStack()
            ctx_ent.__enter__()
            eps = ctx_ent.enter_context(
                tc.tile_pool(name="ent_ps", bufs=2, space="PSUM"))
            epsS = ctx_ent.enter_context(
                tc.tile_pool(name="ent_psS", bufs=1, space="PSUM"))
            for dc in range(5):
                pq = eps.tile([128, SL], F32, tag="pqe")
                for cc in range(5):
                    nc.tensor.matmul(
                        pq[:],
                        wsb["wqe"][:, cc * C + dc * 128:cc * C + (dc + 1) * 128],
                        hqb[:, cc * SL:(cc + 1) * SL],
                        start=(cc == 0), stop=(cc == 4))
                nc.scalar.activation(qeb[:, dc * SL:(dc + 1) * SL], pq[:],
                                     COPY, scale=SCALE_E)
            # phase a: project keT tile-by-tile, scores, exp -> PTe
            PTe = ep.tile([128, 32 * SL], BF16, tag="PTe")
            with tc.tile_pool(name="kep", bufs=3) as kep, \
                 tc.tile_pool(name="ke_ps", bufs=2, space="PSUM") as keps:
                for g in range(8):
                    pse = epsS.tile([128, 4 * SL], F32, tag="pse")
                    for j in range(4):
                        kt = g * 4 + j
                        k0 = kt * 128
                        ke = kep.tile([128, C], BF16, tag="ke")
                        for dc in range(5):
                            pk = keps.tile([128, 128], F32, tag="pk")
                            for cc in range(5):
                                nc.tensor.matmul(
                                    pk[:],
                                    wsb["wke"][:, cc * C + dc * 128:cc * C + (dc + 1) * 128],
                                    hTb[:, cc * S + k0:cc * S + k0 + 128],
                                    start=(cc == 0), stop=(cc == 4))
                            nc.vector.tensor_copy(ke[:, dc * 128:(dc + 1) * 128], pk[:])
                        for dc in range(5):
                            nc.tensor.matmul(
                                pse[:, j * SL:(j + 1) * SL],
                                ke[:, dc * 128:(dc + 1) * 128],
                                qeb[:, dc * SL:(dc + 1) * SL],
                                start=(dc == 0), stop=False)
                        nc.tensor.matmul(pse[:, j * SL:(j + 1) * SL],
                                         augk[:, k0:k0 + 128], augqe[:],
                                         start=False, stop=True)
                    nc.scalar.activation(PTe[:, g * 4 * SL:(g + 1) * 4 * SL],
                                         pse[:], EXP)
            ctx_ent.__exit__(None, None, None)
            # phase b: v projection + AV accumulation (PSUM-resident)
            with tc.tile_pool(name="vkp", bufs=2) as vkp, \
                 tc.tile_pool(name="av_ps", bufs=1, space="PSUM") as avps, \
                 tc.tile_pool(name="vv_ps", bufs=1, space="PSUM") as vvps:
                pave = [avps.tile([128, SL], F32, tag=f"av{dc}", name=f"pave{dc}")
                        for dc in range(5)]
                pden = avps.tile([1, SL], F32, tag="avden")
                for kt in range(32):
                    k0 = kt * 128
                    pv = vvps.tile([128, C], F32, tag="pv")
                    for (o0, w) in ((0, 512), (512, 128)):
                        for cc in range(5):
                            nc.tensor.matmul(
                                pv[:, o0:o0 + w],
                                hTb[:, cc * S + k0:cc * S + k0 + 128],
                                wsb["wve"][:, cc * C + o0:cc * C + o0 + w],
                                start=(cc == 0), stop=(cc == 4))
                    vk = vkp.tile([128, C], BF16, tag="vk")
                    nc.vector.tensor_copy(vk[:], pv[:])
                    for dc in range(5):
                        nc.tensor.matmul(pave[dc][:],
                                         vk[:, dc * 128:(dc + 1) * 128],
                                         PTe[:, kt * SL:(kt + 1) * SL],
                                         start=(kt == 0), stop=(kt == 31))
                    nc.tensor.matmul(pden[:], ones_bf[:],
                                     PTe[:, kt * SL:(kt + 1) * SL],
                                     start=(kt == 0), stop=(kt == 31))
                # evict + normalize via Wo projection with per-partition scale
                oTe = ep.tile([128, 5 * SL], BF16, tag="oTe")
                for dc in range(5):
                    nc.scalar.activation(oTe[:, dc * SL:(dc + 1) * SL],
                                         pave[dc][:], COPY)
                den = ep.tile([1, SL], F32, tag="den")
                nc.vector.tensor_copy(den[:], pden[:])
                rec = ep.tile([1, SL], F32, tag="rec")
                nc.vector.reciprocal(rec[:], den[:])
                recT = base.tile([128, 4], F32, tag="recT")
                for st in range(4):
                    nc.sync.dma_start(recT[:, st:st + 1],
                                      rec[0:1, st * 128:(st + 1) * 128])
            with tc.tile_pool(name="wo_ps", bufs=2, space="PSUM") as wops:
                for st in range(4):
                    pw = wops.tile([128, C], F32, tag="pwoe")
                    for (o0, w) in ((0, 512), (512, 128)):
                        for cc in range(5):
                            nc.tensor.matmul(
                                pw[:, o0:o0 + w],
                                oTe[:, cc * SL + st * 128:cc * SL + (st + 1) * 128],
                                wsb["wof"][:, cc * C + o0:cc * C + o0 + w],
                                start=(cc == 0), stop=(cc == 4))
                    nc.scalar.activation(entout[:, st * C:(st + 1) * C], pw[:],
                                         COPY, scale=recT[:, st:st + 1])

        # ================= orig + out branches (1 head each) ============
        with tc.tile_pool(name="brp", bufs=1) as bp:
            ctx_pj = ExitStack()
            ctx_pj.__enter__()
            pjps = ctx_pj.enter_context(
                tc.tile_pool(name="pj_ps", bufs=2, space="PSUM"))
            tiles = {}
            for br, (wqn, wkn, wvn, aug) in (("o", ("wq", "wk", "wv", augk)),
                                             ("u", ("wqo", "wko", "wvo", augko))):
                qaug = bp.tile([85, S], BF16, tag=f"qaug{br}")
                kaug = bp.tile([85, S], BF16, tag=f"kaug{br}")
                vsb = bp.tile([128, 32 * 97], BF16, tag=f"vsb{br}")
                nc.gpsimd.memset(vsb[:], 1.0)
                for sc in range(8):
                    s0 = sc * 512
                    pq = pjps.tile([80, 512], F32, tag="pq")
                    for cc in range(5):
                        nc.tensor.matmul(pq[:],
                                         wsb[wqn][:, cc * D:(cc + 1) * D],
                                         hTb[:, cc * S + s0:cc * S + s0 + 512],
                                         start=(cc == 0), stop=(cc == 4))
                    nc.scalar.activation(qaug[0:80, s0:s0 + 512], pq[:],
                                         COPY, scale=SCALE_H)
                    pk = pjps.tile([80, 512], F32, tag="pq")
                    for cc in range(5):
                        nc.tensor.matmul(pk[:],
                                         wsb[wkn][:, cc * D:(cc + 1) * D],
                                         hTb[:, cc * S + s0:cc * S + s0 + 512],
                                         start=(cc == 0), stop=(cc == 4))
                    nc.scalar.activation(kaug[0:80, s0:s0 + 512], pk[:], COPY)
                for st in range(32):
                    pv = pjps.tile([128, 80], F32, tag="pv80")
                    for cc in range(5):
                        nc.tensor.matmul(pv[:],
                                         hTb[:, cc * S + st * 128:cc * S + (st + 1) * 128],
                                         wsb[wvn][:, cc * D:(cc + 1) * D],
                                         start=(cc == 0), stop=(cc == 4))
                    nc.vector.tensor_copy(vsb[:, st * 97:st * 97 + 80], pv[:])
                nc.sync.dma_start(qaug[80:85, :], augq[:])
                nc.sync.dma_start(kaug[80:85, :], aug[:])
                tiles[br] = (qaug, kaug, vsb)

            ctx_pj.__exit__(None, None, None)
            # attention (S^T layout, fused mask bias, no-max softmax)
            with tc.tile_pool(name="ptp", bufs=2) as ptp, \
                 tc.tile_pool(name="otp", bufs=2) as otp, \
                 tc.tile_pool(name="at_ps", bufs=1, space="PSUM") as atps, \
                 tc.tile_pool(name="av2_ps", bufs=2, space="PSUM") as av2ps, \
                 tc.tile_pool(name="b_ps", bufs=2, space="PSUM") as bps:
                for br, hsT in (("o", hsTo), ("u", hsTu)):
                    qaug, kaug, vsb = tiles[br]
                    for qc in range(8):
                        q0 = qc * 512
                        pav = av2ps.tile([97, 512], F32, tag="pav")
                        for g in range(8):
                            ps = atps.tile([128, 2048], F32, tag="psS")
                            for j in range(4):
                                kt = g * 4 + j
                                nc.tensor.matmul(ps[:, j * 512:(j + 1) * 512],
                                                 kaug[:, kt * 128:(kt + 1) * 128],
                                                 qaug[:, q0:q0 + 512],
                                                 start=True, stop=True)
                            pt = ptp.tile([128, 2048], BF16, tag="pt")
                            nc.scalar.activation(pt[:], ps[:], EXP)
                            for j in range(4):
                                kt = g * 4 + j
                                nc.tensor.matmul(pav[:],
                                                 vsb[:, kt * 97:kt * 97 + 97],
                                                 pt[:, j * 512:(j + 1) * 512],
                                                 start=(kt == 0), stop=(kt == 31))
                        ot = otp.tile([80, 512], F32, tag="ot")
                        nc.vector.tensor_copy(ot[:], pav[0:80, :])
                        den1 = otp.tile([1, 512], F32, tag="den1")
                        nc.vector.tensor_copy(den1[:], pav[96:97, :])
                        rec2 = otp.tile([1, 512], F32, tag="rec2")
                        nc.vector.reciprocal(rec2[:], den1[:])
                        pB = bps.tile([80, 512], F32, tag="pB")
                        nc.tensor.matmul(pB[:], ones_f[0:1, 0:80], rec2[:],
                                         start=True, stop=True)
                        nc.vector.tensor_mul(hsT[:, q0:q0 + 512],
                                              ot[:], pB[:])

        # ================= Wo partials + ReduceScatter ==================
        with tc.tile_pool(name="wop", bufs=2) as wop, \
             tc.tile_pool(name="wo2_ps", bufs=2, space="PSUM") as wo2ps:
            for st in range(32):
                pw = wo2ps.tile([128, C], F32, tag="pwo")
                for (o0, w) in ((0, 512), (512, 128)):
                    nc.tensor.matmul(pw[:, o0:o0 + w],
                                     hsTo[:, st * 128:(st + 1) * 128],
                                     woh_sb[:, o0:o0 + w],
                                     start=True, stop=False)
                    nc.tensor.matmul(pw[:, o0:o0 + w],
                                     hsTu[:, st * 128:(st + 1) * 128],
                                     woh_sb[:, o0:o0 + w],
                                     start=False, stop=True)
                pb = wop.tile([128, C], F32, tag="pbuf")
                nc.vector.tensor_copy(pb[:], pw[:])
                nc.sync.dma_start(P_dram.ap()[st * 128:(st + 1) * 128, :], pb[:])
            nc.gpsimd.collective_compute(
                "ReduceScatter", ADD,
                replica_groups=[list(range(NCORES))],
                ins=[P_dram.ap()[:]], outs=[Pred_dram.ap()[:]])

        # ================= final: P_red + ent + residual ================
        with tc.tile_pool(name="finp", bufs=1) as fp:
            pred = fp.tile([128, 4 * C], F32, tag="pred")
            resid = fp.tile([128, 4 * C], F32, tag="resid")
            for st in range(4):
                nc.sync.dma_start(pred[:, st * C:(st + 1) * C],
                                  Pred_dram.ap()[st * 128:(st + 1) * 128, :])
                nc.sync.dma_start(resid[:, st * C:(st + 1) * C],
                                  d["res"].ap()[st * 128:(st + 1) * 128, :])
            outt = fp.tile([128, 4 * C], F32, tag="outt")
            nc.vector.tensor_add(outt[:], pred[:], entout[:])
            nc.vector.tensor_add(outt[:], outt[:], resid[:])
            for st in range(4):
                nc.sync.dma_start(out_d.ap()[st * 128:(st + 1) * 128, :],
                                  outt[:, st * C:(st + 1) * C])


def build_in_maps(hidden_states, mask, inpainting_mask, Wq, Wk, Wv,
                  Wq_ent, Wk_ent, Wv_ent, Wq_out, Wk_out, Wv_out, Wo):
    h = np.asarray(hidden_states[0], np.float32)          # [S, C]
    hT = np.ascontiguousarray(h.T)                         # [C, S]
    m2 = np.asarray(mask[0, 0], np.int32)                  # [512, 512]
    im2 = np.asarray(inpainting_mask[0, 0], np.int32)
    mrow = np.ascontiguousarray(m2[::8, ::8]).reshape(1, S)
    imrow = np.ascontiguousarray(im2[::8, ::8]).reshape(1, S)
    WoT = np.ascontiguousarray(np.asarray(Wo, np.float32).T)  # [C, C]

    def wT(W):
        return np.ascontiguousarray(np.asarray(W, np.float32).T)

    in_maps = []
    for i in range(NCORES):
        hd = slice(D * i, D * (i + 1))
        ql = slice(SL * i, SL * (i + 1))
        in_maps.append({
            "hT": hT,
            "hq": np.ascontiguousarray(hT[:, ql]),
            "res": np.ascontiguousarray(h[ql, :]),
            "wq": wT(Wq[hd]), "wk": wT(Wk[hd]), "wv": wT(Wv[hd]),
            "wqo": wT(Wq_out[hd]), "wko": wT(Wk_out[hd]), "wvo": wT(Wv_out[hd]),
            "wqe": wT(Wq_ent), "wke": wT(Wk_ent), "wve": wT(Wv_ent),
            "wof": WoT, "woh": np.ascontiguousarray(WoT[hd, :]),
            "mrow": mrow, "imrow": imrow,
            "mq": np.ascontiguousarray(mrow[:, ql]),
        })
    return in_maps


def kernel(**inputs):
    in_maps = build_in_maps(**inputs)
    if "nc" not in _cache:
        _cache["nc"] = _build()
    res = run_bass_kernel_spmd(_cache["nc"], in_maps, list(range(NCORES)),
                               trace=False)
    out = np.concatenate([res.results[i]["out"] for i in range(NCORES)], axis=0)
    return out.reshape(1, S, C).astype(np.float32)



# revision 11
# speedup vs baseline: 2.8136x; 2.8136x over previous
"""Trainium2 Bass kernel for InpaintingAttnProcessor (3-branch masked SDPA).

Block-sparse formulation: the attention masks depend only on 4 entity
labels, so after sorting tokens by (label, inpainting_bit) on the host,
all three SDPA branches become block-diagonal (the "outside" branch
additionally restricts keys to the im==0 prefix of each block).  Each
core computes one head of the two 8-head branches over all blocks, plus
an entity-aligned slice of the single-head d=640 branch.  Per-entity
bf16 ReduceScatters of the Wo partial products overlap the remaining
compute; the entity branch and the residual are assembled on the host.
"""
import numpy as np
import ml_dtypes
from contextlib import ExitStack

import concourse.bass as bass
import concourse.tile as tile
from concourse import bacc, mybir
from concourse.bass_utils import run_bass_kernel_spmd

S, C, H, D = 4096, 640, 8, 80
NCORES = 8
SCALE_H = 1.0 / np.sqrt(80.0)
SCALE_E = 1.0 / np.sqrt(640.0)
F32 = mybir.dt.float32
BF16 = mybir.dt.bfloat16
BF = ml_dtypes.bfloat16
EXP = mybir.ActivationFunctionType.Exp
COPY = mybir.ActivationFunctionType.Copy
ADD = mybir.AluOpType.add

_cache = {}


def _chunks(total, step=512):
    return [(f0, min(step, total - f0)) for f0 in range(0, total, step)]


def _assign_cores(T):
    """Split entity tiles into NCORES contiguous runs, each within one
    entity. Returns list of (entity, tile0_within_entity, ntiles)."""
    ents = [e for e in range(len(T)) if T[e] > 0]
    c = {e: 1 for e in ents}
    while sum(c.values()) < NCORES:
        e = max(ents, key=lambda x: T[x] / c[x])
        c[e] += 1
    assign = []
    for e in ents:
        base, rem = divmod(T[e], c[e])
        t = 0
        for j in range(c[e]):
            nt = base + (1 if j < rem else 0)
            assign.append((e, t, nt))
            t += nt
    assert len(assign) == NCORES
    return assign


def _build(cfg):
    T, n, n0, assign = cfg
    NE = len(T)
    TEM = max(T)
    NQT = max(a[2] for a in assign)
    Ttot = sum(T)
    Stot = Ttot * 128
    SK, SQ = TEM * 128, NQT * 128
    T0 = [min((x + 127) // 128, T[e]) for e, x in enumerate(n0)]
    off = np.cumsum([0] + [t * 128 for t in T]).tolist()

    nc = bacc.Bacc("TRN2", target_bir_lowering=False, debug=False,
                   num_devices=NCORES)
    d = {}
    d["hT"] = nc.dram_tensor("hT", [C, Stot], BF16, kind="ExternalInput")
    d["whead"] = nc.dram_tensor("whead", [C, 480], BF16, kind="ExternalInput")
    d["went"] = nc.dram_tensor("went", [C, 4 * C], BF16, kind="ExternalInput")
    d["woh"] = nc.dram_tensor("woh", [D, C], BF16, kind="ExternalInput")
    d["hq"] = nc.dram_tensor("hq", [C, SQ], BF16, kind="ExternalInput")
    d["hk"] = nc.dram_tensor("hk", [C, SK], BF16, kind="ExternalInput")
    d["entc"] = nc.dram_tensor("entc", [1, 1], F32, kind="ExternalInput")
    out_d = nc.dram_tensor("out", [Stot // 8, C], BF16, kind="ExternalOutput")
    red_d = nc.dram_tensor("red", [Stot // 8, C], BF16)
    eout_d = nc.dram_tensor("eout", [SQ, C], BF16, kind="ExternalOutput")
    P_d = [nc.dram_tensor(f"P{e}", [T[e] * 128, C], BF16) if T[e] else None
           for e in range(NE)]

    with tile.TileContext(nc) as tc:
        _body(nc, tc, d, out_d, red_d, eout_d, P_d, T, T0, n, n0, off,
              TEM, NQT)
    nc.compile()
    return nc


def _body(nc, tc, d, out_d, red_d, eout_d, P_d, T, T0, n, n0, off, TEM,
          NQT):
    NE = len(T)
    Ttot = sum(T)
    Stot = Ttot * 128
    SK, SQ = TEM * 128, NQT * 128
    W4 = 4 * C                      # went row width
    ctx = ExitStack()
    with ctx:
        base = ctx.enter_context(tc.tile_pool(name="base", bufs=1))
        hTb = base.tile([128, 5 * Stot], BF16, tag="hTb")
        wh = base.tile([128, 5 * 480], BF16, tag="wh")
        woh_sb = base.tile([D, C], BF16, tag="woh")
        ones_bf = base.tile([128, 1], BF16, tag="ones_bf")
        ones_f = base.tile([1, 128], F32, tag="ones_f")
        entc_sb = base.tile([1, 1], F32, tag="entc")
        nc.vector.memset(ones_bf[:], 1.0)
        nc.vector.memset(ones_f[:], 1.0)
        nc.sync.dma_start(entc_sb[:], d["entc"].ap()[:])
        nc.sync.dma_start(woh_sb[:], d["woh"].ap()[:])
        for cc in range(5):
            nc.sync.dma_start(wh[:, cc * 480:(cc + 1) * 480],
                              d["whead"].ap()[cc * 128:(cc + 1) * 128, :])

        # ================= ENT branch (entity-aligned q slice) ==========
        with tc.tile_pool(name="entp", bufs=1) as ep:
            went = ep.tile([128, 5 * W4], BF16, tag="went")
            hqb = ep.tile([128, 5 * SQ], BF16, tag="hqb")
            hkb = ep.tile([128, 5 * SK], BF16, tag="hkb")
            for cc in range(5):
                nc.sync.dma_start(went[:, cc * W4:(cc + 1) * W4],
                                  d["went"].ap()[cc * 128:(cc + 1) * 128, :])
                nc.sync.dma_start(hqb[:, cc * SQ:(cc + 1) * SQ],
                                  d["hq"].ap()[cc * 128:(cc + 1) * 128, :])
                nc.sync.dma_start(hkb[:, cc * SK:(cc + 1) * SK],
                                  d["hk"].ap()[cc * 128:(cc + 1) * 128, :])
            # hT load queued after the small ent inputs
            for cc in range(5):
                nc.sync.dma_start(hTb[:, cc * Stot:(cc + 1) * Stot],
                                  d["hT"].ap()[cc * 128:(cc + 1) * 128, :])

            qeb = ep.tile([128, 5 * SQ], BF16, tag="qeb")
            keb = ep.tile([128, 5 * SK], BF16, tag="keb")
            veb = ep.tile([128, TEM * C], BF16, tag="veb")
            with tc.tile_pool(name="entps", bufs=2, space="PSUM") as eps:
                for dc in range(5):
                    for f0, fw in _chunks(SQ):
                        pp = eps.tile([128, C], F32, tag="pp")
                        for cc in range(5):
                            nc.tensor.matmul(
                                pp[:, 0:fw],
                                went[:, cc * W4 + dc * 128:cc * W4 + (dc + 1) * 128],
                                hqb[:, cc * SQ + f0:cc * SQ + f0 + fw],
                                start=(cc == 0), stop=(cc == 4))
                        nc.vector.tensor_copy(
                            qeb[:, dc * SQ + f0:dc * SQ + f0 + fw], pp[:, 0:fw])
                for dc in range(5):
                    for f0, fw in _chunks(SK):
                        pp = eps.tile([128, C], F32, tag="pp")
                        for cc in range(5):
                            nc.tensor.matmul(
                                pp[:, 0:fw],
                                went[:, cc * W4 + C + dc * 128:cc * W4 + C + (dc + 1) * 128],
                                hkb[:, cc * SK + f0:cc * SK + f0 + fw],
                                start=(cc == 0), stop=(cc == 4))
                        nc.vector.tensor_copy(
                            keb[:, dc * SK + f0:dc * SK + f0 + fw], pp[:, 0:fw])
                for kt in range(TEM):
                    pp = eps.tile([128, C], F32, tag="pp")
                    for o0, w in ((0, 512), (512, 128)):
                        for cc in range(5):
                            nc.tensor.matmul(
                                pp[:, o0:o0 + w],
                                hkb[:, cc * SK + kt * 128:cc * SK + (kt + 1) * 128],
                                went[:, cc * W4 + 2 * C + o0:cc * W4 + 2 * C + o0 + w],
                                start=(cc == 0), stop=(cc == 4))
                    nc.vector.tensor_copy(veb[:, kt * C:(kt + 1) * C], pp[:])

            oTe = ep.tile([128, 5 * SQ], BF16, tag="oTe")
            PTe = ep.tile([128, TEM * 512], BF16, tag="PTe")
            den_s = ep.tile([1, 512], F32, tag="den_s")
            rec_s = ep.tile([1, 512], F32, tag="rec_s")
            for q0, qw in _chunks(SQ):
                with tc.tile_pool(name="entsc", bufs=2, space="PSUM") as scp, \
                     tc.tile_pool(name="entav", bufs=1, space="PSUM") as avp:
                    pave = avp.tile([128, 5 * 512], F32, tag="pave")
                    pden = avp.tile([1, 512], F32, tag="pden")
                    for kt in range(TEM):
                        pse = scp.tile([128, 512], F32, tag="pse")
                        for dc in range(5):
                            nc.tensor.matmul(
                                pse[:, 0:qw],
                                keb[:, dc * SK + kt * 128:dc * SK + (kt + 1) * 128],
                                qeb[:, dc * SQ + q0:dc * SQ + q0 + qw],
                                start=(dc == 0), stop=(dc == 4))
                        nc.scalar.activation(PTe[:, kt * qw:(kt + 1) * qw],
                                             pse[:, 0:qw], EXP)
                    for kt in range(TEM):
                        for dc in range(5):
                            # dc*512: one PSUM bank per concurrent accum group
                            nc.tensor.matmul(
                                pave[:, dc * 512:dc * 512 + qw],
                                veb[:, kt * C + dc * 128:kt * C + (dc + 1) * 128],
                                PTe[:, kt * qw:(kt + 1) * qw],
                                start=(kt == 0), stop=(kt == TEM - 1))
                        nc.tensor.matmul(pden[:, 0:qw], ones_bf[:],
                                         PTe[:, kt * qw:(kt + 1) * qw],
                                         start=(kt == 0), stop=(kt == TEM - 1))
                    nc.vector.tensor_scalar(den_s[0:1, 0:qw], pden[:, 0:qw],
                                            entc_sb[0:1, 0:1], None, op0=ADD)
                    nc.vector.reciprocal(rec_s[0:1, 0:qw], den_s[0:1, 0:qw])
                    pB = scp.tile([128, 512], F32, tag="pse", name="pB")
                    nc.tensor.matmul(pB[:, 0:qw], ones_f[0:1, :],
                                     rec_s[0:1, 0:qw], start=True, stop=True)
                    pBs = ep.tile([128, 512], F32, tag="pBs")
                    nc.vector.tensor_copy(pBs[:, 0:qw], pB[:, 0:qw])
                    for dc in range(5):
                        nc.vector.tensor_mul(
                            oTe[:, dc * SQ + q0:dc * SQ + q0 + qw],
                            pave[:, dc * 512:dc * 512 + qw], pBs[:, 0:qw])
            # ent Wo projection -> eout
            eoutb = ep.tile([128, NQT * C], BF16, tag="eoutb")
            with tc.tile_pool(name="entwo", bufs=2, space="PSUM") as ewp:
                for st in range(NQT):
                    pw = ewp.tile([128, C], F32, tag="pwe")
                    for o0, w in ((0, 512), (512, 128)):
                        for cc in range(5):
                            nc.tensor.matmul(
                                pw[:, o0:o0 + w],
                                oTe[:, cc * SQ + st * 128:cc * SQ + (st + 1) * 128],
                                went[:, cc * W4 + 3 * C + o0:cc * W4 + 3 * C + o0 + w],
                                start=(cc == 0), stop=(cc == 4))
                    nc.scalar.activation(eoutb[:, st * C:(st + 1) * C], pw[:],
                                         COPY)
                for st in range(NQT):
                    nc.sync.dma_start(eout_d.ap()[st * 128:(st + 1) * 128, :],
                                      eoutb[:, st * C:(st + 1) * C])

        # ============ orig + out branches (1 head each per core) ========
        main = ctx.enter_context(tc.tile_pool(name="main", bufs=1))
        qTo = main.tile([D, Stot], BF16, tag="qTo")
        kTo = main.tile([D, Stot], BF16, tag="kTo")
        qTu = main.tile([D, Stot], BF16, tag="qTu")
        kTu = main.tile([D, Stot], BF16, tag="kTu")
        vso = main.tile([128, Ttot * 97], BF16, tag="vso")
        vsu = main.tile([128, Ttot * 97], BF16, tag="vsu")
        hsTo = main.tile([D, Stot], BF16, tag="hsTo")
        hsTu = main.tile([D, Stot], BF16, tag="hsTu")
        PT = main.tile([128, TEM * 512], BF16, tag="PT")
        nc.gpsimd.memset(vso[:], 1.0)
        nc.gpsimd.memset(vsu[:], 1.0)

        with tc.tile_pool(name="pjps", bufs=2, space="PSUM") as pjp:
            for dst, wcol, scl in ((qTo, 0, True), (kTo, 80, False),
                                   (qTu, 160, True), (kTu, 240, False)):
                for f0, fw in _chunks(Stot):
                    pq = pjp.tile([D, 512], F32, tag="pq")
                    for cc in range(5):
                        nc.tensor.matmul(
                            pq[:, 0:fw],
                            wh[:, cc * 480 + wcol:cc * 480 + wcol + D],
                            hTb[:, cc * Stot + f0:cc * Stot + f0 + fw],
                            start=(cc == 0), stop=(cc == 4))
                    nc.vector.tensor_copy(dst[:, f0:f0 + fw], pq[:, 0:fw])
            # out-branch boundary tiles: keys n0[e]..T0[e]*128 are im==1 and
            # must not contribute -> zero their k columns and v rows
            bnd = {}
            for e in range(NE):
                if T[e] == 0 or n0[e] == 0 or n0[e] % 128 == 0:
                    continue
                bnd[off[e] // 128 + T0[e] - 1] = n0[e] % 128
                nc.vector.memset(kTu[:, off[e] + n0[e]:off[e] + T0[e] * 128],
                                 0.0)
            for kt in range(Ttot):
                pv = pjp.tile([128, 160], F32, tag="pv")
                for cc in range(5):
                    nc.tensor.matmul(
                        pv[:],
                        hTb[:, cc * Stot + kt * 128:cc * Stot + (kt + 1) * 128],
                        wh[:, cc * 480 + 320:cc * 480 + 480],
                        start=(cc == 0), stop=(cc == 4))
                nc.vector.tensor_copy(vso[:, kt * 97:kt * 97 + 80], pv[:, 0:80])
                if kt in bnd:
                    nc.vector.memset(vsu[:, kt * 97:kt * 97 + 80], 0.0)
                    nc.vector.tensor_copy(vsu[0:bnd[kt], kt * 97:kt * 97 + 80],
                                          pv[0:bnd[kt], 80:160])
                else:
                    nc.vector.tensor_copy(vsu[:, kt * 97:kt * 97 + 80],
                                          pv[:, 80:160])

        # attention + Wo partials + per-entity ReduceScatter
        atx = ExitStack()
        with atx:
            psp = atx.enter_context(tc.tile_pool(name="psp", bufs=2, space="PSUM"))
            avp = atx.enter_context(tc.tile_pool(name="avp", bufs=2, space="PSUM"))
            wop = atx.enter_context(tc.tile_pool(name="wop", bufs=1, space="PSUM"))
            sb2 = atx.enter_context(tc.tile_pool(name="sb2", bufs=2))
            outoff = 0
            for e in range(NE):
                if T[e] == 0:
                    continue
                oe = off[e]
                for br, qT, kT, vs, hsT, nkt, corr in (
                        ("o", qTo, kTo, vso, hsTo, T[e],
                         float(n[e] - T[e] * 128)),
                        ("u", qTu, kTu, vsu, hsTu, T0[e],
                         float(n0[e] - T0[e] * 128))):
                    for q0, qw in _chunks(T[e] * 128):
                        # scores + exp for all key tiles of this block
                        for g0 in range(0, nkt, 2):
                            gn = min(2, nkt - g0)
                            ps = psp.tile([128, 1024], F32, tag="ps")
                            for j in range(gn):
                                kt = g0 + j
                                nc.tensor.matmul(
                                    ps[:, j * qw:j * qw + qw],
                                    kT[:, oe + kt * 128:oe + (kt + 1) * 128],
                                    qT[:, oe + q0:oe + q0 + qw],
                                    start=True, stop=True)
                            nc.scalar.activation(
                                PT[:, g0 * qw:(g0 + gn) * qw],
                                ps[:, 0:gn * qw], EXP)
                        pav = avp.tile([128, 512], F32, tag="pav")
                        for kt in range(nkt):
                            nc.tensor.matmul(
                                pav[0:97, 0:qw],
                                vs[:, (oe // 128 + kt) * 97:(oe // 128 + kt) * 97 + 97],
                                PT[:, kt * qw:(kt + 1) * qw],
                                start=(kt == 0), stop=(kt == nkt - 1))
                        dn0 = sb2.tile([1, 512], F32, tag="dn0")
                        dn = sb2.tile([1, 512], F32, tag="dn")
                        rc = sb2.tile([1, 512], F32, tag="rc")
                        nc.vector.tensor_copy(dn0[0:1, 0:qw], pav[96:97, 0:qw])
                        nc.vector.tensor_scalar(dn[0:1, 0:qw], dn0[0:1, 0:qw],
                                                corr, None, op0=ADD)
                        nc.vector.reciprocal(rc[0:1, 0:qw], dn[0:1, 0:qw])
                        pB = psp.tile([128, 1024], F32, tag="ps", name="pBm")
                        nc.tensor.matmul(pB[0:D, 0:qw], ones_f[0:1, 0:D],
                                         rc[0:1, 0:qw], start=True, stop=True)
                        pBs = sb2.tile([D, 512], F32, tag="pBs")
                        nc.vector.tensor_copy(pBs[:, 0:qw], pB[0:D, 0:qw])
                        nc.vector.tensor_mul(hsT[:, oe + q0:oe + q0 + qw],
                                             pav[0:D, 0:qw], pBs[:, 0:qw])
                # Wo partials for this entity's rows, then ReduceScatter
                for st in range(T[e]):
                    gt = oe // 128 + st
                    pw = wop.tile([128, C], F32, tag="pw")
                    for o0, w in ((0, 512), (512, 128)):
                        nc.tensor.matmul(pw[:, o0:o0 + w],
                                         hsTo[:, gt * 128:(gt + 1) * 128],
                                         woh_sb[:, o0:o0 + w],
                                         start=True, stop=False)
                        nc.tensor.matmul(pw[:, o0:o0 + w],
                                         hsTu[:, gt * 128:(gt + 1) * 128],
                                         woh_sb[:, o0:o0 + w],
                                         start=False, stop=True)
                    pbuf = sb2.tile([128, C], BF16, tag="pbuf")
                    nc.scalar.activation(pbuf[:], pw[:], COPY)
                    nc.sync.dma_start(P_d[e].ap()[st * 128:(st + 1) * 128, :],
                                      pbuf[:])
                ne8 = T[e] * 16
                nc.gpsimd.collective_compute(
                    "ReduceScatter", ADD,
                    replica_groups=[list(range(NCORES))],
                    ins=[P_d[e].ap()[:]],
                    outs=[red_d.ap()[outoff:outoff + ne8, :]])
                outoff += ne8
            # bounce the reduced result through SBUF into the IO tensor
            for r0 in range(0, Stot // 8, 128):
                rw = min(128, Stot // 8 - r0)
                rb = sb2.tile([128, C], BF16, tag="rb")
                nc.sync.dma_start(rb[0:rw, :], red_d.ap()[r0:r0 + rw, :])
                nc.sync.dma_start(out_d.ap()[r0:r0 + rw, :], rb[0:rw, :])


def _plan(mask, inpainting_mask):
    m = np.asarray(mask[0, 0], np.int64)[::8, ::8].reshape(-1)
    im = np.asarray(inpainting_mask[0, 0], np.int64)[::8, ::8].reshape(-1)
    NE = int(m.max()) + 1
    n = [int((m == e).sum()) for e in range(NE)]
    n0 = [int(((m == e) & (im == 0)).sum()) for e in range(NE)]
    for e in range(NE):
        assert n[e] == 0 or n0[e] > 0, "empty outside-key block unsupported"
    T = [(x + 127) // 128 for x in n]
    order = np.lexsort((im, m))
    off = np.cumsum([0] + [t * 128 for t in T])
    pos = np.concatenate([off[e] + np.arange(n[e]) for e in range(NE)
                          if n[e] > 0]).astype(np.int64)
    assign = tuple(_assign_cores(T))
    cfg = (tuple(T), tuple(n), tuple(n0), assign)
    return cfg, order, pos, off


def build_in_maps(hidden_states, mask, inpainting_mask, Wq, Wk, Wv,
                  Wq_ent, Wk_ent, Wv_ent, Wq_out, Wk_out, Wv_out, Wo):
    cfg, order, pos, off = _plan(mask, inpainting_mask)
    T, n, n0, assign = cfg
    TEM = max(T)
    NQT = max(a[2] for a in assign)
    Stot = sum(T) * 128
    SK, SQ = TEM * 128, NQT * 128

    h = np.asarray(hidden_states[0], np.float32)
    hp = np.zeros((Stot, C), np.float32)
    hp[pos] = h[order]
    hTb = np.ascontiguousarray(hp.T).astype(BF)

    def t(W):
        return np.asarray(W, np.float32).T

    went = np.ascontiguousarray(np.concatenate(
        [t(Wq_ent) * SCALE_E, t(Wk_ent), t(Wv_ent), t(Wo)], axis=1)).astype(BF)
    WoT = t(Wo)

    in_maps = []
    for i in range(NCORES):
        hd = slice(D * i, D * (i + 1))
        whead = np.ascontiguousarray(np.concatenate(
            [t(Wq)[:, hd] * SCALE_H, t(Wk)[:, hd],
             t(Wq_out)[:, hd] * SCALE_H, t(Wk_out)[:, hd],
             t(Wv)[:, hd], t(Wv_out)[:, hd]], axis=1)).astype(BF)
        e, t0, nt = assign[i]
        hq = np.zeros((C, SQ), BF)
        hq[:, :nt * 128] = hTb[:, off[e] + t0 * 128:off[e] + (t0 + nt) * 128]
        hk = np.zeros((C, SK), BF)
        hk[:, :T[e] * 128] = hTb[:, off[e]:off[e] + T[e] * 128]
        in_maps.append({
            "hT": hTb, "whead": whead, "went": went,
            "woh": np.ascontiguousarray(WoT[hd, :]).astype(BF),
            "hq": hq, "hk": hk,
            "entc": np.array([[n[e] - TEM * 128]], np.float32),
        })
    _cache["plan"] = (cfg, order, pos, off, hp)
    return in_maps


def kernel(**inputs):
    in_maps = build_in_maps(**inputs)
    cfg, order, pos, off, hp = _cache["plan"]
    T, n, n0, assign = cfg
    Stot = sum(T) * 128
    key = ("nc", cfg)
    if key not in _cache:
        _cache["nc"] = _build(cfg)
        _cache[key] = _cache["nc"]
    res = run_bass_kernel_spmd(_cache[key], in_maps, list(range(NCORES)),
                               trace=False)
    acc = np.zeros((Stot, C), np.float32)
    outoff = 0
    for e in range(len(T)):
        if T[e] == 0:
            continue
        ne8 = T[e] * 16
        for i in range(NCORES):
            acc[off[e] + i * ne8:off[e] + (i + 1) * ne8] = \
                np.asarray(res.results[i]["out"][outoff:outoff + ne8],
                           np.float32)
        outoff += ne8
    for i, (e, t0, nt) in enumerate(assign):
        q0 = off[e] + t0 * 128
        acc[q0:q0 + nt * 128] += np.asarray(
            res.results[i]["eout"][:nt * 128], np.float32)
    acc += hp
    out = np.empty((S, C), np.float32)
    out[order] = acc[pos]
    return out.reshape(1, S, C)


# revision 17
# speedup vs baseline: 3.1780x; 1.1295x over previous
"""Trainium2 Bass kernel for InpaintingAttnProcessor (3-branch masked SDPA).

Block-sparse formulation: the attention masks depend only on 4 entity
labels, so after sorting tokens by (label, inpainting_bit) on the host,
all three SDPA branches become block-diagonal (the "outside" branch
additionally restricts keys to the im==0 prefix of each block).  Each
core computes one head of the two 8-head branches over all blocks, plus
an entity-aligned slice of the single-head d=640 branch.  Per-entity
bf16 ReduceScatters of the Wo partial products overlap the remaining
compute; the entity branch and the residual are assembled on the host.
"""
import numpy as np
import ml_dtypes
from contextlib import ExitStack

import concourse.bass as bass
import concourse.tile as tile
from concourse import bacc, mybir
from concourse.bass_utils import run_bass_kernel_spmd

S, C, H, D = 4096, 640, 8, 80
NCORES = 8
SCALE_H = 1.0 / np.sqrt(80.0)
SCALE_E = 1.0 / np.sqrt(640.0)
F32 = mybir.dt.float32
BF16 = mybir.dt.bfloat16
BF = ml_dtypes.bfloat16
EXP = mybir.ActivationFunctionType.Exp
COPY = mybir.ActivationFunctionType.Copy
ADD = mybir.AluOpType.add

_cache = {}


def _chunks(total, step=512):
    return [(f0, min(step, total - f0)) for f0 in range(0, total, step)]


def _assign_cores(T):
    """Split entity tiles into NCORES contiguous runs, each within one
    entity. Returns list of (entity, tile0_within_entity, ntiles)."""
    ents = [e for e in range(len(T)) if T[e] > 0]
    c = {e: 1 for e in ents}
    while sum(c.values()) < NCORES:
        e = max(ents, key=lambda x: T[x] / c[x])
        c[e] += 1
    assign = []
    for e in ents:
        base, rem = divmod(T[e], c[e])
        t = 0
        for j in range(c[e]):
            nt = base + (1 if j < rem else 0)
            assign.append((e, t, nt))
            t += nt
    assert len(assign) == NCORES
    return assign


def _build(cfg):
    T, n, n0, assign = cfg
    NE = len(T)
    TEM = max(T)
    NQT = max(a[2] for a in assign)
    Ttot = sum(T)
    Stot = Ttot * 128
    SK, SQ = TEM * 128, NQT * 128
    T0 = [min((x + 127) // 128, T[e]) for e, x in enumerate(n0)]
    off = np.cumsum([0] + [t * 128 for t in T]).tolist()

    nc = bacc.Bacc("TRN2", target_bir_lowering=False, debug=False,
                   num_devices=NCORES)
    d = {}
    d["hT"] = nc.dram_tensor("hT", [C, Stot], BF16, kind="ExternalInput")
    d["whead"] = nc.dram_tensor("whead", [C, 480], BF16, kind="ExternalInput")
    d["went"] = nc.dram_tensor("went", [C, 4 * C], BF16, kind="ExternalInput")
    d["woh"] = nc.dram_tensor("woh", [D, C], BF16, kind="ExternalInput")
    d["hq"] = nc.dram_tensor("hq", [C, SQ], BF16, kind="ExternalInput")
    d["hk"] = nc.dram_tensor("hk", [C, SK], BF16, kind="ExternalInput")
    d["entc"] = nc.dram_tensor("entc", [1, 1], F32, kind="ExternalInput")
    out_d = nc.dram_tensor("out", [Stot // 8, C], BF16, kind="ExternalOutput")
    red_d = nc.dram_tensor("red", [Stot // 8, C], BF16)
    eout_d = nc.dram_tensor("eout", [SQ, C], BF16, kind="ExternalOutput")
    P_d = [nc.dram_tensor(f"P{e}", [T[e] * 128, C], BF16) if T[e] else None
           for e in range(NE)]

    with tile.TileContext(nc) as tc:
        _body(nc, tc, d, out_d, red_d, eout_d, P_d, T, T0, n, n0, off,
              TEM, NQT)
    nc.compile()
    return nc


def _body(nc, tc, d, out_d, red_d, eout_d, P_d, T, T0, n, n0, off, TEM,
          NQT):
    NE = len(T)
    Ttot = sum(T)
    Stot = Ttot * 128
    SK, SQ = TEM * 128, NQT * 128
    W4 = 4 * C                      # went row width
    ctx = ExitStack()
    with ctx:
        base = ctx.enter_context(tc.tile_pool(name="base", bufs=1))
        hTb = base.tile([128, 5 * Stot], BF16, tag="hTb")
        wh = base.tile([128, 5 * 480], BF16, tag="wh")
        woh_sb = base.tile([D, C], BF16, tag="woh")
        ones_bf = base.tile([128, 1], BF16, tag="ones_bf")
        ones_f = base.tile([1, 128], F32, tag="ones_f")
        entc_sb = base.tile([1, 1], F32, tag="entc")
        nc.vector.memset(ones_bf[:], 1.0)
        nc.vector.memset(ones_f[:], 1.0)
        nc.sync.dma_start(entc_sb[:], d["entc"].ap()[:])
        nc.sync.dma_start(woh_sb[:], d["woh"].ap()[:])
        for cc in range(5):
            nc.sync.dma_start(wh[:, cc * 480:(cc + 1) * 480],
                              d["whead"].ap()[cc * 128:(cc + 1) * 128, :])

        # ================= ENT branch (entity-aligned q slice) ==========
        with tc.tile_pool(name="entp", bufs=1) as ep:
            went = ep.tile([128, 5 * W4], BF16, tag="went")
            hqb = ep.tile([128, 5 * SQ], BF16, tag="hqb")
            hkb = ep.tile([128, 5 * SK], BF16, tag="hkb")
            # load order: q-proj operands first so the PE can start early,
            # then k, v, wof, then the big hT tensor
            for cc in range(5):
                nc.sync.dma_start(
                    went[:, cc * W4:cc * W4 + C],
                    d["went"].ap()[cc * 128:(cc + 1) * 128, 0:C])
                nc.sync.dma_start(hqb[:, cc * SQ:(cc + 1) * SQ],
                                  d["hq"].ap()[cc * 128:(cc + 1) * 128, :])
            for cc in range(5):
                nc.sync.dma_start(
                    went[:, cc * W4 + C:cc * W4 + 2 * C],
                    d["went"].ap()[cc * 128:(cc + 1) * 128, C:2 * C])
                nc.sync.dma_start(hkb[:, cc * SK:(cc + 1) * SK],
                                  d["hk"].ap()[cc * 128:(cc + 1) * 128, :])
            for cc in range(5):
                nc.sync.dma_start(
                    went[:, cc * W4 + 2 * C:(cc + 1) * W4],
                    d["went"].ap()[cc * 128:(cc + 1) * 128, 2 * C:W4])
            for cc in range(5):
                nc.sync.dma_start(hTb[:, cc * Stot:(cc + 1) * Stot],
                                  d["hT"].ap()[cc * 128:(cc + 1) * 128, :])

            qeb = ep.tile([128, 5 * SQ], BF16, tag="qeb")
            keb = ep.tile([128, 5 * SK], BF16, tag="keb")
            veb = ep.tile([128, TEM * C], BF16, tag="veb")
            with tc.tile_pool(name="entps", bufs=2, space="PSUM") as eps:
                for dc in range(5):
                    for f0, fw in _chunks(SQ):
                        pp = eps.tile([128, C], F32, tag="pp")
                        for cc in range(5):
                            nc.tensor.matmul(
                                pp[:, 0:fw],
                                went[:, cc * W4 + dc * 128:cc * W4 + (dc + 1) * 128],
                                hqb[:, cc * SQ + f0:cc * SQ + f0 + fw],
                                start=(cc == 0), stop=(cc == 4))
                        nc.vector.tensor_copy(
                            qeb[:, dc * SQ + f0:dc * SQ + f0 + fw], pp[:, 0:fw])
                for dc in range(5):
                    for f0, fw in _chunks(SK):
                        pp = eps.tile([128, C], F32, tag="pp")
                        for cc in range(5):
                            nc.tensor.matmul(
                                pp[:, 0:fw],
                                went[:, cc * W4 + C + dc * 128:cc * W4 + C + (dc + 1) * 128],
                                hkb[:, cc * SK + f0:cc * SK + f0 + fw],
                                start=(cc == 0), stop=(cc == 4))
                        nc.vector.tensor_copy(
                            keb[:, dc * SK + f0:dc * SK + f0 + fw], pp[:, 0:fw])
                for kt in range(TEM):
                    pp = eps.tile([128, C], F32, tag="pp")
                    for o0, w in ((0, 512), (512, 128)):
                        for cc in range(5):
                            nc.tensor.matmul(
                                pp[:, o0:o0 + w],
                                hkb[:, cc * SK + kt * 128:cc * SK + (kt + 1) * 128],
                                went[:, cc * W4 + 2 * C + o0:cc * W4 + 2 * C + o0 + w],
                                start=(cc == 0), stop=(cc == 4))
                    nc.vector.tensor_copy(veb[:, kt * C:(kt + 1) * C], pp[:])

            oTe = ep.tile([128, 5 * SQ], BF16, tag="oTe")
            PTe = ep.tile([128, TEM * 512], BF16, tag="PTe")
            den_s = ep.tile([1, 512], F32, tag="den_s")
            rec_s = ep.tile([1, 512], F32, tag="rec_s")
            for q0, qw in _chunks(SQ):
                with tc.tile_pool(name="entsc", bufs=2, space="PSUM") as scp, \
                     tc.tile_pool(name="entav", bufs=1, space="PSUM") as avp:
                    pave = avp.tile([128, 5 * 512], F32, tag="pave")
                    pden = avp.tile([1, 512], F32, tag="pden")
                    for kt in range(TEM):
                        pse = scp.tile([128, 512], F32, tag="pse")
                        for dc in range(5):
                            nc.tensor.matmul(
                                pse[:, 0:qw],
                                keb[:, dc * SK + kt * 128:dc * SK + (kt + 1) * 128],
                                qeb[:, dc * SQ + q0:dc * SQ + q0 + qw],
                                start=(dc == 0), stop=(dc == 4))
                        nc.scalar.activation(PTe[:, kt * qw:(kt + 1) * qw],
                                             pse[:, 0:qw], EXP)
                    for kt in range(TEM):
                        for dc in range(5):
                            # dc*512: one PSUM bank per concurrent accum group
                            nc.tensor.matmul(
                                pave[:, dc * 512:dc * 512 + qw],
                                veb[:, kt * C + dc * 128:kt * C + (dc + 1) * 128],
                                PTe[:, kt * qw:(kt + 1) * qw],
                                start=(kt == 0), stop=(kt == TEM - 1))
                        nc.tensor.matmul(pden[:, 0:qw], ones_bf[:],
                                         PTe[:, kt * qw:(kt + 1) * qw],
                                         start=(kt == 0), stop=(kt == TEM - 1))
                    nc.vector.tensor_scalar(den_s[0:1, 0:qw], pden[:, 0:qw],
                                            entc_sb[0:1, 0:1], None, op0=ADD)
                    nc.vector.reciprocal_approx_fast(rec_s[0:1, 0:qw],
                                                     den_s[0:1, 0:qw])
                    pB = scp.tile([128, 512], F32, tag="pse", name="pB")
                    nc.tensor.matmul(pB[:, 0:qw], ones_f[0:1, :],
                                     rec_s[0:1, 0:qw], start=True, stop=True)
                    pBs = ep.tile([128, 512], F32, tag="pBs")
                    nc.vector.tensor_copy(pBs[:, 0:qw], pB[:, 0:qw])
                    for dc in range(5):
                        nc.vector.tensor_mul(
                            oTe[:, dc * SQ + q0:dc * SQ + q0 + qw],
                            pave[:, dc * 512:dc * 512 + qw], pBs[:, 0:qw])
            # ent Wo projection -> eout
            eoutb = ep.tile([128, NQT * C], BF16, tag="eoutb")
            with tc.tile_pool(name="entwo", bufs=2, space="PSUM") as ewp:
                for st in range(NQT):
                    pw = ewp.tile([128, C], F32, tag="pwe")
                    for o0, w in ((0, 512), (512, 128)):
                        for cc in range(5):
                            nc.tensor.matmul(
                                pw[:, o0:o0 + w],
                                oTe[:, cc * SQ + st * 128:cc * SQ + (st + 1) * 128],
                                went[:, cc * W4 + 3 * C + o0:cc * W4 + 3 * C + o0 + w],
                                start=(cc == 0), stop=(cc == 4))
                    nc.scalar.activation(eoutb[:, st * C:(st + 1) * C], pw[:],
                                         COPY)
                for st in range(NQT):
                    nc.sync.dma_start(eout_d.ap()[st * 128:(st + 1) * 128, :],
                                      eoutb[:, st * C:(st + 1) * C])

        # ============ orig + out branches (1 head each per core) ========
        main = ctx.enter_context(tc.tile_pool(name="main", bufs=1))
        qTo = main.tile([D, Stot], BF16, tag="qTo")
        kTo = main.tile([D, Stot], BF16, tag="kTo")
        qTu = main.tile([D, Stot], BF16, tag="qTu")
        kTu = main.tile([D, Stot], BF16, tag="kTu")
        vso = main.tile([128, Ttot * 97], BF16, tag="vso")
        vsu = main.tile([128, Ttot * 97], BF16, tag="vsu")
        hsTo = main.tile([D, Stot], BF16, tag="hsTo")
        hsTu = main.tile([D, Stot], BF16, tag="hsTu")
        hsTs = main.tile([D, Stot], BF16, tag="hsTs")
        PTs = [main.tile([128, TEM * 512], BF16, tag="PT0", name="PT0"),
               main.tile([128, TEM * 512], BF16, tag="PT1", name="PT1")]
        nc.gpsimd.memset(vso[:], 1.0)
        nc.gpsimd.memset(vsu[:], 1.0)

        with tc.tile_pool(name="pjps", bufs=2, space="PSUM") as pjp:
            for dst, wcol, scl in ((qTo, 0, True), (kTo, 80, False),
                                   (qTu, 160, True), (kTu, 240, False)):
                for f0, fw in _chunks(Stot):
                    pq = pjp.tile([D, 512], F32, tag="pq")
                    for cc in range(5):
                        nc.tensor.matmul(
                            pq[:, 0:fw],
                            wh[:, cc * 480 + wcol:cc * 480 + wcol + D],
                            hTb[:, cc * Stot + f0:cc * Stot + f0 + fw],
                            start=(cc == 0), stop=(cc == 4))
                    nc.vector.tensor_copy(dst[:, f0:f0 + fw], pq[:, 0:fw])
            # out-branch boundary tiles: keys n0[e]..T0[e]*128 are im==1 and
            # must not contribute -> zero their k columns and v rows
            bnd = {}
            for e in range(NE):
                if T[e] == 0 or n0[e] == 0 or n0[e] % 128 == 0:
                    continue
                bnd[off[e] // 128 + T0[e] - 1] = n0[e] % 128
                nc.vector.memset(kTu[:, off[e] + n0[e]:off[e] + T0[e] * 128],
                                 0.0)
            for kt in range(Ttot):
                pv = pjp.tile([128, 160], F32, tag="pv")
                for cc in range(5):
                    nc.tensor.matmul(
                        pv[:],
                        hTb[:, cc * Stot + kt * 128:cc * Stot + (kt + 1) * 128],
                        wh[:, cc * 480 + 320:cc * 480 + 480],
                        start=(cc == 0), stop=(cc == 4))
                nc.vector.tensor_copy(vso[:, kt * 97:kt * 97 + 80], pv[:, 0:80])
                if kt in bnd:
                    nc.vector.memset(vsu[:, kt * 97:kt * 97 + 80], 0.0)
                    nc.vector.tensor_copy(vsu[0:bnd[kt], kt * 97:kt * 97 + 80],
                                          pv[0:bnd[kt], 80:160])
                else:
                    nc.vector.tensor_copy(vsu[:, kt * 97:kt * 97 + 80],
                                          pv[:, 80:160])

        # attention + Wo partials + per-entity ReduceScatter
        atx = ExitStack()
        with atx:
            psp = atx.enter_context(tc.tile_pool(name="psp", bufs=2, space="PSUM"))
            avp = atx.enter_context(tc.tile_pool(name="avp", bufs=2, space="PSUM"))
            wop = atx.enter_context(tc.tile_pool(name="wop", bufs=1, space="PSUM"))
            sb2 = atx.enter_context(tc.tile_pool(name="sb2", bufs=2))
            outoff = 0
            ci = 0
            eorder = sorted([e for e in range(NE) if T[e] > 0],
                            key=lambda e: (-T[e], e))
            for e in eorder:
                oe = off[e]
                for br, qT, kT, vs, hsT, nkt, corr in (
                        ("o", qTo, kTo, vso, hsTo, T[e],
                         float(n[e] - T[e] * 128)),
                        ("u", qTu, kTu, vsu, hsTu, T0[e],
                         float(n0[e] - T0[e] * 128))):
                    for q0, qw in _chunks(T[e] * 128):
                        PT = PTs[ci % 2]
                        ci += 1
                        # scores + exp for all key tiles of this block
                        for g0 in range(0, nkt, 2):
                            gn = min(2, nkt - g0)
                            ps = psp.tile([128, 1024], F32, tag="ps")
                            for j in range(gn):
                                kt = g0 + j
                                nc.tensor.matmul(
                                    ps[:, j * qw:j * qw + qw],
                                    kT[:, oe + kt * 128:oe + (kt + 1) * 128],
                                    qT[:, oe + q0:oe + q0 + qw],
                                    start=True, stop=True)
                            nc.scalar.activation(
                                PT[:, g0 * qw:(g0 + gn) * qw],
                                ps[:, 0:gn * qw], EXP)
                        pav = avp.tile([128, 512], F32, tag="pav")
                        for kt in range(nkt):
                            nc.tensor.matmul(
                                pav[0:97, 0:qw],
                                vs[:, (oe // 128 + kt) * 97:(oe // 128 + kt) * 97 + 97],
                                PT[:, kt * qw:(kt + 1) * qw],
                                start=(kt == 0), stop=(kt == nkt - 1))
                        dn0 = sb2.tile([1, 512], F32, tag="dn0")
                        dn = sb2.tile([1, 512], F32, tag="dn")
                        rc = sb2.tile([1, 512], F32, tag="rc")
                        nc.vector.tensor_copy(dn0[0:1, 0:qw], pav[96:97, 0:qw])
                        nc.vector.tensor_scalar(dn[0:1, 0:qw], dn0[0:1, 0:qw],
                                                corr, None, op0=ADD)
                        nc.vector.reciprocal_approx_fast(rc[0:1, 0:qw],
                                                         dn[0:1, 0:qw])
                        pB = psp.tile([128, 1024], F32, tag="ps", name="pBm")
                        nc.tensor.matmul(pB[0:D, 0:qw], ones_f[0:1, 0:D],
                                         rc[0:1, 0:qw], start=True, stop=True)
                        pBs = sb2.tile([D, 512], F32, tag="pBs")
                        nc.vector.tensor_copy(pBs[:, 0:qw], pB[0:D, 0:qw])
                        nc.vector.tensor_mul(hsT[:, oe + q0:oe + q0 + qw],
                                             pav[0:D, 0:qw], pBs[:, 0:qw])
                        if br == "u":
                            nc.vector.tensor_add(
                                hsTs[:, oe + q0:oe + q0 + qw],
                                hsTo[:, oe + q0:oe + q0 + qw],
                                hsTu[:, oe + q0:oe + q0 + qw])
                # Wo partials for this entity's rows, then ReduceScatter
                for st in range(T[e]):
                    gt = oe // 128 + st
                    pw = wop.tile([128, C], F32, tag="pw")
                    for o0, w in ((0, 512), (512, 128)):
                        nc.tensor.matmul(pw[:, o0:o0 + w],
                                         hsTs[:, gt * 128:(gt + 1) * 128],
                                         woh_sb[:, o0:o0 + w],
                                         start=True, stop=True)
                    pbuf = sb2.tile([128, C], BF16, tag="pbuf")
                    nc.scalar.activation(pbuf[:], pw[:], COPY)
                    nc.sync.dma_start(P_d[e].ap()[st * 128:(st + 1) * 128, :],
                                      pbuf[:])
                ne8 = T[e] * 16
                nc.gpsimd.collective_compute(
                    "ReduceScatter", ADD,
                    replica_groups=[list(range(NCORES))],
                    ins=[P_d[e].ap()[:]],
                    outs=[red_d.ap()[outoff:outoff + ne8, :]])
                outoff += ne8
            # bounce the reduced result through SBUF into the IO tensor
            for r0 in range(0, Stot // 8, 128):
                rw = min(128, Stot // 8 - r0)
                rb = sb2.tile([128, C], BF16, tag="rb")
                nc.sync.dma_start(rb[0:rw, :], red_d.ap()[r0:r0 + rw, :])
                nc.sync.dma_start(out_d.ap()[r0:r0 + rw, :], rb[0:rw, :])


def _plan(mask, inpainting_mask):
    m = np.asarray(mask[0, 0], np.int64)[::8, ::8].reshape(-1)
    im = np.asarray(inpainting_mask[0, 0], np.int64)[::8, ::8].reshape(-1)
    NE = int(m.max()) + 1
    n = [int((m == e).sum()) for e in range(NE)]
    n0 = [int(((m == e) & (im == 0)).sum()) for e in range(NE)]
    for e in range(NE):
        assert n[e] == 0 or n0[e] > 0, "empty outside-key block unsupported"
    T = [(x + 127) // 128 for x in n]
    order = np.lexsort((im, m))
    off = np.cumsum([0] + [t * 128 for t in T])
    pos = np.concatenate([off[e] + np.arange(n[e]) for e in range(NE)
                          if n[e] > 0]).astype(np.int64)
    assign = tuple(_assign_cores(T))
    cfg = (tuple(T), tuple(n), tuple(n0), assign)
    return cfg, order, pos, off


def build_in_maps(hidden_states, mask, inpainting_mask, Wq, Wk, Wv,
                  Wq_ent, Wk_ent, Wv_ent, Wq_out, Wk_out, Wv_out, Wo):
    cfg, order, pos, off = _plan(mask, inpainting_mask)
    T, n, n0, assign = cfg
    TEM = max(T)
    NQT = max(a[2] for a in assign)
    Stot = sum(T) * 128
    SK, SQ = TEM * 128, NQT * 128

    h = np.asarray(hidden_states[0], np.float32)
    hp = np.zeros((Stot, C), np.float32)
    hp[pos] = h[order]
    hTb = np.ascontiguousarray(hp.T).astype(BF)

    def t(W):
        return np.asarray(W, np.float32).T

    went = np.ascontiguousarray(np.concatenate(
        [t(Wq_ent) * SCALE_E, t(Wk_ent), t(Wv_ent), t(Wo)], axis=1)).astype(BF)
    WoT = t(Wo)

    in_maps = []
    for i in range(NCORES):
        hd = slice(D * i, D * (i + 1))
        whead = np.ascontiguousarray(np.concatenate(
            [t(Wq)[:, hd] * SCALE_H, t(Wk)[:, hd],
             t(Wq_out)[:, hd] * SCALE_H, t(Wk_out)[:, hd],
             t(Wv)[:, hd], t(Wv_out)[:, hd]], axis=1)).astype(BF)
        e, t0, nt = assign[i]
        hq = np.zeros((C, SQ), BF)
        hq[:, :nt * 128] = hTb[:, off[e] + t0 * 128:off[e] + (t0 + nt) * 128]
        hk = np.zeros((C, SK), BF)
        hk[:, :T[e] * 128] = hTb[:, off[e]:off[e] + T[e] * 128]
        in_maps.append({
            "hT": hTb, "whead": whead, "went": went,
            "woh": np.ascontiguousarray(WoT[hd, :]).astype(BF),
            "hq": hq, "hk": hk,
            "entc": np.array([[n[e] - TEM * 128]], np.float32),
        })
    _cache["plan"] = (cfg, order, pos, off, hp)
    return in_maps


def kernel(**inputs):
    in_maps = build_in_maps(**inputs)
    cfg, order, pos, off, hp = _cache["plan"]
    T, n, n0, assign = cfg
    Stot = sum(T) * 128
    key = ("nc", cfg)
    if key not in _cache:
        _cache["nc"] = _build(cfg)
        _cache[key] = _cache["nc"]
    res = run_bass_kernel_spmd(_cache[key], in_maps, list(range(NCORES)),
                               trace=False)
    acc = np.zeros((Stot, C), np.float32)
    outoff = 0
    eorder = sorted([e for e in range(len(T)) if T[e] > 0],
                    key=lambda e: (-T[e], e))
    for e in eorder:
        ne8 = T[e] * 16
        for i in range(NCORES):
            acc[off[e] + i * ne8:off[e] + (i + 1) * ne8] = \
                np.asarray(res.results[i]["out"][outoff:outoff + ne8],
                           np.float32)
        outoff += ne8
    for i, (e, t0, nt) in enumerate(assign):
        q0 = off[e] + t0 * 128
        acc[q0:q0 + nt * 128] += np.asarray(
            res.results[i]["eout"][:nt * 128], np.float32)
    acc += hp
    out = np.empty((S, C), np.float32)
    out[order] = acc[pos]
    return out.reshape(1, S, C)


# revision 22
# speedup vs baseline: 3.4899x; 1.0981x over previous
"""Trainium2 Bass kernel for InpaintingAttnProcessor (3-branch masked SDPA).

Block-sparse formulation: the attention masks depend only on 4 entity
labels, so after sorting tokens by (label, inpainting_bit) on the host,
all three SDPA branches become block-diagonal (the "outside" branch
additionally restricts keys to the im==0 prefix of each block).  Each
core computes one head of the two 8-head branches over all blocks, plus
an entity-aligned slice of the single-head d=640 branch.  Per-entity
bf16 ReduceScatters of the Wo partial products overlap the remaining
compute; the entity branch and the residual are assembled on the host.
"""
import numpy as np
import ml_dtypes
from contextlib import ExitStack

import concourse.bass as bass
import concourse.tile as tile
from concourse import bacc, mybir
from concourse.bass_utils import run_bass_kernel_spmd

S, C, H, D = 4096, 640, 8, 80
NCORES = 8
SCALE_H = 1.0 / np.sqrt(80.0)
SCALE_E = 1.0 / np.sqrt(640.0)
F32 = mybir.dt.float32
BF16 = mybir.dt.bfloat16
BF = ml_dtypes.bfloat16
EXP = mybir.ActivationFunctionType.Exp
COPY = mybir.ActivationFunctionType.Copy
ADD = mybir.AluOpType.add

_cache = {}


def _chunks(total, step=512):
    return [(f0, min(step, total - f0)) for f0 in range(0, total, step)]


def _assign_cores(T):
    """Split entity tiles into NCORES contiguous runs, each within one
    entity. Returns list of (entity, tile0_within_entity, ntiles)."""
    ents = [e for e in range(len(T)) if T[e] > 0]
    c = {e: 1 for e in ents}
    while sum(c.values()) < NCORES:
        e = max(ents, key=lambda x: T[x] / c[x])
        c[e] += 1
    assign = []
    for e in ents:
        base, rem = divmod(T[e], c[e])
        t = 0
        for j in range(c[e]):
            nt = base + (1 if j < rem else 0)
            assign.append((e, t, nt))
            t += nt
    assert len(assign) == NCORES
    return assign


def _build(cfg):
    T, n, n0, assign = cfg
    NE = len(T)
    TEM = max(T)
    NQT = max(a[2] for a in assign)
    Ttot = sum(T)
    Stot = Ttot * 128
    SK, SQ = TEM * 128, NQT * 128
    T0 = [min((x + 127) // 128, T[e]) for e, x in enumerate(n0)]
    off = np.cumsum([0] + [t * 128 for t in T]).tolist()

    nc = bacc.Bacc("TRN2", target_bir_lowering=False, debug=False,
                   num_devices=NCORES)
    d = {}
    d["hT"] = nc.dram_tensor("hT", [C, Stot], BF16, kind="ExternalInput")
    d["whead"] = nc.dram_tensor("whead", [C, 480], BF16, kind="ExternalInput")
    d["went"] = nc.dram_tensor("went", [C, 4 * C], BF16, kind="ExternalInput")
    d["woh"] = nc.dram_tensor("woh", [D, C], BF16, kind="ExternalInput")
    d["hq"] = nc.dram_tensor("hq", [C, SQ], BF16, kind="ExternalInput")
    d["hk"] = nc.dram_tensor("hk", [C, SK], BF16, kind="ExternalInput")
    d["entc"] = nc.dram_tensor("entc", [1, 1], F32, kind="ExternalInput")
    out_d = nc.dram_tensor("out", [Stot // 8, C], BF16, kind="ExternalOutput")
    red_d = nc.dram_tensor("red", [Stot // 8, C], BF16)
    eout_d = nc.dram_tensor("eout", [SQ, C], BF16, kind="ExternalOutput")
    P_d = [nc.dram_tensor(f"P{e}", [T[e] * 128, C], BF16) if T[e] else None
           for e in range(NE)]

    with tile.TileContext(nc) as tc:
        _body(nc, tc, d, out_d, red_d, eout_d, P_d, T, T0, n, n0, off,
              TEM, NQT)
    nc.compile()
    return nc


def _body(nc, tc, d, out_d, red_d, eout_d, P_d, T, T0, n, n0, off, TEM,
          NQT):
    NE = len(T)
    Ttot = sum(T)
    Stot = Ttot * 128
    SK, SQ = TEM * 128, NQT * 128
    W4 = 4 * C                      # went row width
    ctx = ExitStack()
    with ctx:
        base = ctx.enter_context(tc.tile_pool(name="base", bufs=1))
        hTb = base.tile([128, 5 * Stot], BF16, tag="hTb")
        wh = base.tile([128, 5 * 480], BF16, tag="wh")
        woh_sb = base.tile([D, C], BF16, tag="woh")
        ones_bf = base.tile([128, 1], BF16, tag="ones_bf")
        ones_f = base.tile([1, 128], F32, tag="ones_f")
        ones_b1 = base.tile([1, 128], BF16, tag="ones_b1")
        entc_sb = base.tile([1, 1], F32, tag="entc")
        nc.vector.memset(ones_bf[:], 1.0)
        nc.vector.memset(ones_f[:], 1.0)
        nc.vector.memset(ones_b1[:], 1.0)
        nc.sync.dma_start(entc_sb[:], d["entc"].ap()[:])
        nc.sync.dma_start(woh_sb[:], d["woh"].ap()[:])
        for cc in range(5):
            nc.sync.dma_start(wh[:, cc * 480:(cc + 1) * 480],
                              d["whead"].ap()[cc * 128:(cc + 1) * 128, :])

        # ================= ENT branch (entity-aligned q slice) ==========
        with tc.tile_pool(name="entp", bufs=1) as ep:
            went = ep.tile([128, 5 * W4], BF16, tag="went")
            hqb = ep.tile([128, 5 * SQ], BF16, tag="hqb")
            hkb = ep.tile([128, 5 * SK], BF16, tag="hkb")
            # load order: q-proj operands first so the PE can start early,
            # then k, v, wof, then the big hT tensor
            for cc in range(5):
                nc.sync.dma_start(
                    went[:, cc * W4:cc * W4 + C],
                    d["went"].ap()[cc * 128:(cc + 1) * 128, 0:C])
                nc.sync.dma_start(hqb[:, cc * SQ:(cc + 1) * SQ],
                                  d["hq"].ap()[cc * 128:(cc + 1) * 128, :])
            for cc in range(5):
                nc.sync.dma_start(
                    went[:, cc * W4 + C:cc * W4 + 2 * C],
                    d["went"].ap()[cc * 128:(cc + 1) * 128, C:2 * C])
                nc.sync.dma_start(hkb[:, cc * SK:(cc + 1) * SK],
                                  d["hk"].ap()[cc * 128:(cc + 1) * 128, :])
            for cc in range(5):
                nc.sync.dma_start(
                    went[:, cc * W4 + 2 * C:(cc + 1) * W4],
                    d["went"].ap()[cc * 128:(cc + 1) * 128, 2 * C:W4])
            for cc in range(5):
                nc.sync.dma_start(hTb[:, cc * Stot:(cc + 1) * Stot],
                                  d["hT"].ap()[cc * 128:(cc + 1) * 128, :])

            qeb = ep.tile([128, 5 * SQ], BF16, tag="qeb")
            keb = ep.tile([128, 5 * SK], BF16, tag="keb")
            veb = ep.tile([128, TEM * C], BF16, tag="veb")
            with tc.tile_pool(name="entps", bufs=2, space="PSUM") as eps:
                for dc in range(5):
                    for f0, fw in _chunks(SQ):
                        pp = eps.tile([128, C], F32, tag="pp")
                        for cc in range(5):
                            nc.tensor.matmul(
                                pp[:, 0:fw],
                                went[:, cc * W4 + dc * 128:cc * W4 + (dc + 1) * 128],
                                hqb[:, cc * SQ + f0:cc * SQ + f0 + fw],
                                start=(cc == 0), stop=(cc == 4))
                        nc.vector.tensor_copy(
                            qeb[:, dc * SQ + f0:dc * SQ + f0 + fw], pp[:, 0:fw])
                for dc in range(5):
                    for f0, fw in _chunks(SK):
                        pp = eps.tile([128, C], F32, tag="pp")
                        for cc in range(5):
                            nc.tensor.matmul(
                                pp[:, 0:fw],
                                went[:, cc * W4 + C + dc * 128:cc * W4 + C + (dc + 1) * 128],
                                hkb[:, cc * SK + f0:cc * SK + f0 + fw],
                                start=(cc == 0), stop=(cc == 4))
                        nc.vector.tensor_copy(
                            keb[:, dc * SK + f0:dc * SK + f0 + fw], pp[:, 0:fw])
                for kt in range(TEM):
                    pp = eps.tile([128, C], F32, tag="pp")
                    for o0, w in ((0, 512), (512, 128)):
                        for cc in range(5):
                            nc.tensor.matmul(
                                pp[:, o0:o0 + w],
                                hkb[:, cc * SK + kt * 128:cc * SK + (kt + 1) * 128],
                                went[:, cc * W4 + 2 * C + o0:cc * W4 + 2 * C + o0 + w],
                                start=(cc == 0), stop=(cc == 4))
                    nc.vector.tensor_copy(veb[:, kt * C:(kt + 1) * C], pp[:])

            oTe = ep.tile([128, 5 * SQ], BF16, tag="oTe")
            PTe = ep.tile([128, TEM * 512], BF16, tag="PTe")
            den_s = ep.tile([1, 512], F32, tag="den_s")
            rec_s = ep.tile([1, 512], F32, tag="rec_s")
            for q0, qw in _chunks(SQ):
                with tc.tile_pool(name="entsc", bufs=2, space="PSUM") as scp, \
                     tc.tile_pool(name="entav", bufs=1, space="PSUM") as avp:
                    pave = avp.tile([128, 5 * 512], F32, tag="pave")
                    pden = avp.tile([1, 512], F32, tag="pden")
                    for kt in range(TEM):
                        pse = scp.tile([128, 512], F32, tag="pse")
                        for dc in range(5):
                            nc.tensor.matmul(
                                pse[:, 0:qw],
                                keb[:, dc * SK + kt * 128:dc * SK + (kt + 1) * 128],
                                qeb[:, dc * SQ + q0:dc * SQ + q0 + qw],
                                start=(dc == 0), stop=(dc == 4))
                        nc.scalar.activation(PTe[:, kt * qw:(kt + 1) * qw],
                                             pse[:, 0:qw], EXP)
                    for kt in range(TEM):
                        for dc in range(5):
                            # dc*512: one PSUM bank per concurrent accum group
                            nc.tensor.matmul(
                                pave[:, dc * 512:dc * 512 + qw],
                                veb[:, kt * C + dc * 128:kt * C + (dc + 1) * 128],
                                PTe[:, kt * qw:(kt + 1) * qw],
                                start=(kt == 0), stop=(kt == TEM - 1))
                        nc.tensor.matmul(pden[:, 0:qw], ones_bf[:],
                                         PTe[:, kt * qw:(kt + 1) * qw],
                                         start=(kt == 0), stop=(kt == TEM - 1))
                    nc.vector.tensor_scalar(den_s[0:1, 0:qw], pden[:, 0:qw],
                                            entc_sb[0:1, 0:1], None, op0=ADD)
                    nc.vector.reciprocal_approx_fast(rec_s[0:1, 0:qw],
                                                     den_s[0:1, 0:qw])
                    rec_b = ep.tile([1, 512], BF16, tag="rec_b")
                    nc.vector.tensor_copy(rec_b[0:1, 0:qw], rec_s[0:1, 0:qw])
                    pB = scp.tile([128, 512], F32, tag="pse", name="pB")
                    nc.tensor.matmul(pB[:, 0:qw], ones_b1[0:1, :],
                                     rec_b[0:1, 0:qw], start=True, stop=True)
                    pBs = ep.tile([128, 512], F32, tag="pBs")
                    nc.vector.tensor_copy(pBs[:, 0:qw], pB[:, 0:qw])
                    for dc in range(5):
                        nc.vector.tensor_mul(
                            oTe[:, dc * SQ + q0:dc * SQ + q0 + qw],
                            pave[:, dc * 512:dc * 512 + qw], pBs[:, 0:qw])
            # ent Wo projection -> eout
            eoutb = ep.tile([128, NQT * C], BF16, tag="eoutb")
            with tc.tile_pool(name="entwo", bufs=2, space="PSUM") as ewp:
                for st in range(NQT):
                    pw = ewp.tile([128, C], F32, tag="pwe")
                    for o0, w in ((0, 512), (512, 128)):
                        for cc in range(5):
                            nc.tensor.matmul(
                                pw[:, o0:o0 + w],
                                oTe[:, cc * SQ + st * 128:cc * SQ + (st + 1) * 128],
                                went[:, cc * W4 + 3 * C + o0:cc * W4 + 3 * C + o0 + w],
                                start=(cc == 0), stop=(cc == 4))
                    nc.scalar.activation(eoutb[:, st * C:(st + 1) * C], pw[:],
                                         COPY)
                for st in range(NQT):
                    nc.sync.dma_start(eout_d.ap()[st * 128:(st + 1) * 128, :],
                                      eoutb[:, st * C:(st + 1) * C])

        # ============ orig + out branches (1 head each per core) ========
        main = ctx.enter_context(tc.tile_pool(name="main", bufs=1))
        qTo = main.tile([D, Stot], BF16, tag="qTo")
        kTo = main.tile([D, Stot], BF16, tag="kTo")
        qTu = main.tile([D, Stot], BF16, tag="qTu")
        kTu = main.tile([D, Stot], BF16, tag="kTu")
        vso = main.tile([128, Ttot * 97], BF16, tag="vso")
        vsu = main.tile([128, Ttot * 97], BF16, tag="vsu")
        hsTo = main.tile([D, Stot], BF16, tag="hsTo")
        hsTu = main.tile([D, Stot], BF16, tag="hsTu")
        hsTs = main.tile([D, Stot], BF16, tag="hsTs")
        PTs = [main.tile([128, TEM * 512], BF16, tag="PT0", name="PT0"),
               main.tile([128, TEM * 512], BF16, tag="PT1", name="PT1")]
        nc.gpsimd.memset(vso[:], 1.0)
        nc.gpsimd.memset(vsu[:], 1.0)

        with tc.tile_pool(name="pjps", bufs=2, space="PSUM") as pjp:
            for dst, wcol, scl in ((qTo, 0, True), (kTo, 80, False),
                                   (qTu, 160, True), (kTu, 240, False)):
                for f0, fw in _chunks(Stot):
                    pq = pjp.tile([D, 512], F32, tag="pq")
                    for cc in range(5):
                        nc.tensor.matmul(
                            pq[:, 0:fw],
                            wh[:, cc * 480 + wcol:cc * 480 + wcol + D],
                            hTb[:, cc * Stot + f0:cc * Stot + f0 + fw],
                            start=(cc == 0), stop=(cc == 4))
                    nc.vector.tensor_copy(dst[:, f0:f0 + fw], pq[:, 0:fw])
            # out-branch boundary tiles: keys n0[e]..T0[e]*128 are im==1 and
            # must not contribute -> zero their k columns and v rows
            bnd = {}
            for e in range(NE):
                if T[e] == 0 or n0[e] == 0 or n0[e] % 128 == 0:
                    continue
                bnd[off[e] // 128 + T0[e] - 1] = n0[e] % 128
                nc.vector.memset(kTu[:, off[e] + n0[e]:off[e] + T0[e] * 128],
                                 0.0)
            for kt in range(Ttot):
                pv = pjp.tile([128, 160], F32, tag="pv")
                for cc in range(5):
                    nc.tensor.matmul(
                        pv[:],
                        hTb[:, cc * Stot + kt * 128:cc * Stot + (kt + 1) * 128],
                        wh[:, cc * 480 + 320:cc * 480 + 480],
                        start=(cc == 0), stop=(cc == 4))
                nc.vector.tensor_copy(vso[:, kt * 97:kt * 97 + 80], pv[:, 0:80])
                if kt in bnd:
                    nc.vector.memset(vsu[:, kt * 97:kt * 97 + 80], 0.0)
                    nc.vector.tensor_copy(vsu[0:bnd[kt], kt * 97:kt * 97 + 80],
                                          pv[0:bnd[kt], 80:160])
                else:
                    nc.vector.tensor_copy(vsu[:, kt * 97:kt * 97 + 80],
                                          pv[:, 80:160])

        # attention + Wo partials + per-entity ReduceScatter
        atx = ExitStack()
        with atx:
            psp = atx.enter_context(tc.tile_pool(name="psp", bufs=2, space="PSUM"))
            avp = atx.enter_context(tc.tile_pool(name="avp", bufs=2, space="PSUM"))
            wop = atx.enter_context(tc.tile_pool(name="wop", bufs=1, space="PSUM"))
            sb2 = atx.enter_context(tc.tile_pool(name="sb2", bufs=2))
            # deep pool so Wo evictions never wait for P-write DMAs that
            # are queued behind a running collective
            pbp = atx.enter_context(tc.tile_pool(name="pbp", bufs=10))
            outoff = 0
            ci = 0
            eorder = sorted([e for e in range(NE) if T[e] > 0],
                            key=lambda e: (-T[e], e))
            for e in eorder:
                oe = off[e]
                for br, qT, kT, vs, hsT, nkt, corr in (
                        ("o", qTo, kTo, vso, hsTo, T[e],
                         float(n[e] - T[e] * 128)),
                        ("u", qTu, kTu, vsu, hsTu, T0[e],
                         float(n0[e] - T0[e] * 128))):
                    for q0, qw in _chunks(T[e] * 128):
                        PT = PTs[ci % 2]
                        ci += 1
                        # scores + exp for all key tiles of this block
                        for g0 in range(0, nkt, 2):
                            gn = min(2, nkt - g0)
                            ps = psp.tile([128, 1024], F32, tag="ps")
                            for j in range(gn):
                                kt = g0 + j
                                nc.tensor.matmul(
                                    ps[:, j * qw:j * qw + qw],
                                    kT[:, oe + kt * 128:oe + (kt + 1) * 128],
                                    qT[:, oe + q0:oe + q0 + qw],
                                    start=True, stop=True)
                            nc.scalar.activation(
                                PT[:, g0 * qw:(g0 + gn) * qw],
                                ps[:, 0:gn * qw], EXP)
                        pav = avp.tile([128, 512], F32, tag="pav")
                        for kt in range(nkt):
                            nc.tensor.matmul(
                                pav[0:97, 0:qw],
                                vs[:, (oe // 128 + kt) * 97:(oe // 128 + kt) * 97 + 97],
                                PT[:, kt * qw:(kt + 1) * qw],
                                start=(kt == 0), stop=(kt == nkt - 1))
                        dn = sb2.tile([1, 512], F32, tag="dn")
                        rc = sb2.tile([1, 512], F32, tag="rc")
                        rcb = sb2.tile([1, 512], BF16, tag="rcb")
                        nc.vector.tensor_scalar(dn[0:1, 0:qw],
                                                pav[96:97, 0:qw],
                                                corr, None, op0=ADD)
                        nc.vector.reciprocal_approx_fast(rc[0:1, 0:qw],
                                                         dn[0:1, 0:qw])
                        nc.vector.tensor_copy(rcb[0:1, 0:qw], rc[0:1, 0:qw])
                        pB = psp.tile([128, 1024], F32, tag="ps", name="pBm")
                        nc.tensor.matmul(pB[0:D, 0:qw], ones_b1[0:1, 0:D],
                                         rcb[0:1, 0:qw], start=True, stop=True)
                        pBs = sb2.tile([D, 512], F32, tag="pBs")
                        nc.vector.tensor_copy(pBs[:, 0:qw], pB[0:D, 0:qw])
                        nc.vector.tensor_mul(hsT[:, oe + q0:oe + q0 + qw],
                                             pav[0:D, 0:qw], pBs[:, 0:qw])
                        if br == "u":
                            nc.vector.tensor_add(
                                hsTs[:, oe + q0:oe + q0 + qw],
                                hsTo[:, oe + q0:oe + q0 + qw],
                                hsTu[:, oe + q0:oe + q0 + qw])
                # Wo partials for this entity's rows, then ReduceScatter
                for st in range(T[e]):
                    gt = oe // 128 + st
                    pw = wop.tile([128, C], F32, tag="pw")
                    for o0, w in ((0, 512), (512, 128)):
                        nc.tensor.matmul(pw[:, o0:o0 + w],
                                         hsTs[:, gt * 128:(gt + 1) * 128],
                                         woh_sb[:, o0:o0 + w],
                                         start=True, stop=True)
                    pbuf = pbp.tile([128, C], BF16, tag="pbuf")
                    nc.scalar.activation(pbuf[:], pw[:], COPY)
                    nc.sync.dma_start(P_d[e].ap()[st * 128:(st + 1) * 128, :],
                                      pbuf[:])
                ne8 = T[e] * 16
                nc.gpsimd.collective_compute(
                    "ReduceScatter", ADD,
                    replica_groups=[list(range(NCORES))],
                    ins=[P_d[e].ap()[:]],
                    outs=[red_d.ap()[outoff:outoff + ne8, :]])
                outoff += ne8
            # bounce the reduced result through SBUF into the IO tensor
            for r0 in range(0, Stot // 8, 128):
                rw = min(128, Stot // 8 - r0)
                rb = sb2.tile([128, C], BF16, tag="rb")
                nc.sync.dma_start(rb[0:rw, :], red_d.ap()[r0:r0 + rw, :])
                nc.sync.dma_start(out_d.ap()[r0:r0 + rw, :], rb[0:rw, :])


def _plan(mask, inpainting_mask):
    m = np.asarray(mask[0, 0], np.int64)[::8, ::8].reshape(-1)
    im = np.asarray(inpainting_mask[0, 0], np.int64)[::8, ::8].reshape(-1)
    NE = int(m.max()) + 1
    n = [int((m == e).sum()) for e in range(NE)]
    n0 = [int(((m == e) & (im == 0)).sum()) for e in range(NE)]
    for e in range(NE):
        assert n[e] == 0 or n0[e] > 0, "empty outside-key block unsupported"
    T = [(x + 127) // 128 for x in n]
    order = np.lexsort((im, m))
    off = np.cumsum([0] + [t * 128 for t in T])
    pos = np.concatenate([off[e] + np.arange(n[e]) for e in range(NE)
                          if n[e] > 0]).astype(np.int64)
    assign = tuple(_assign_cores(T))
    cfg = (tuple(T), tuple(n), tuple(n0), assign)
    return cfg, order, pos, off


def build_in_maps(hidden_states, mask, inpainting_mask, Wq, Wk, Wv,
                  Wq_ent, Wk_ent, Wv_ent, Wq_out, Wk_out, Wv_out, Wo):
    cfg, order, pos, off = _plan(mask, inpainting_mask)
    T, n, n0, assign = cfg
    TEM = max(T)
    NQT = max(a[2] for a in assign)
    Stot = sum(T) * 128
    SK, SQ = TEM * 128, NQT * 128

    h = np.asarray(hidden_states[0], np.float32)
    hp = np.zeros((Stot, C), np.float32)
    hp[pos] = h[order]
    hTb = np.ascontiguousarray(hp.T).astype(BF)

    def t(W):
        return np.asarray(W, np.float32).T

    went = np.ascontiguousarray(np.concatenate(
        [t(Wq_ent) * SCALE_E, t(Wk_ent), t(Wv_ent), t(Wo)], axis=1)).astype(BF)
    WoT = t(Wo)

    in_maps = []
    for i in range(NCORES):
        hd = slice(D * i, D * (i + 1))
        whead = np.ascontiguousarray(np.concatenate(
            [t(Wq)[:, hd] * SCALE_H, t(Wk)[:, hd],
             t(Wq_out)[:, hd] * SCALE_H, t(Wk_out)[:, hd],
             t(Wv)[:, hd], t(Wv_out)[:, hd]], axis=1)).astype(BF)
        e, t0, nt = assign[i]
        hq = np.zeros((C, SQ), BF)
        hq[:, :nt * 128] = hTb[:, off[e] + t0 * 128:off[e] + (t0 + nt) * 128]
        hk = np.zeros((C, SK), BF)
        hk[:, :T[e] * 128] = hTb[:, off[e]:off[e] + T[e] * 128]
        in_maps.append({
            "hT": hTb, "whead": whead, "went": went,
            "woh": np.ascontiguousarray(WoT[hd, :]).astype(BF),
            "hq": hq, "hk": hk,
            "entc": np.array([[n[e] - TEM * 128]], np.float32),
        })
    _cache["plan"] = (cfg, order, pos, off, hp)
    return in_maps


def kernel(**inputs):
    in_maps = build_in_maps(**inputs)
    cfg, order, pos, off, hp = _cache["plan"]
    T, n, n0, assign = cfg
    Stot = sum(T) * 128
    key = ("nc", cfg)
    if key not in _cache:
        _cache["nc"] = _build(cfg)
        _cache[key] = _cache["nc"]
    res = run_bass_kernel_spmd(_cache[key], in_maps, list(range(NCORES)),
                               trace=False)
    acc = np.zeros((Stot, C), np.float32)
    outoff = 0
    eorder = sorted([e for e in range(len(T)) if T[e] > 0],
                    key=lambda e: (-T[e], e))
    for e in eorder:
        ne8 = T[e] * 16
        for i in range(NCORES):
            acc[off[e] + i * ne8:off[e] + (i + 1) * ne8] = \
                np.asarray(res.results[i]["out"][outoff:outoff + ne8],
                           np.float32)
        outoff += ne8
    for i, (e, t0, nt) in enumerate(assign):
        q0 = off[e] + t0 * 128
        acc[q0:q0 + nt * 128] += np.asarray(
            res.results[i]["eout"][:nt * 128], np.float32)
    acc += hp
    out = np.empty((S, C), np.float32)
    out[order] = acc[pos]
    return out.reshape(1, S, C)


# revision 23
# speedup vs baseline: 3.8308x; 1.0977x over previous
"""Trainium2 Bass kernel for InpaintingAttnProcessor (3-branch masked SDPA).

Block-sparse formulation: the attention masks depend only on 4 entity
labels, so after sorting tokens by (label, inpainting_bit) on the host,
all three SDPA branches become block-diagonal (the "outside" branch
additionally restricts keys to the im==0 prefix of each block).  Each
core computes one head of the two 8-head branches over all blocks, plus
an entity-aligned slice of the single-head d=640 branch.  Per-entity
bf16 ReduceScatters of the Wo partial products overlap the remaining
compute; the entity branch and the residual are assembled on the host.
"""
import numpy as np
import ml_dtypes
from contextlib import ExitStack

import concourse.bass as bass
import concourse.tile as tile
from concourse import bacc, mybir
from concourse.bass_utils import run_bass_kernel_spmd

S, C, H, D = 4096, 640, 8, 80
NCORES = 8
SCALE_H = 1.0 / np.sqrt(80.0)
SCALE_E = 1.0 / np.sqrt(640.0)
F32 = mybir.dt.float32
BF16 = mybir.dt.bfloat16
BF = ml_dtypes.bfloat16
EXP = mybir.ActivationFunctionType.Exp
COPY = mybir.ActivationFunctionType.Copy
ADD = mybir.AluOpType.add

_cache = {}


def _chunks(total, step=512):
    return [(f0, min(step, total - f0)) for f0 in range(0, total, step)]


def _assign_cores(T):
    """Split entity tiles into NCORES contiguous runs, each within one
    entity. Returns list of (entity, tile0_within_entity, ntiles)."""
    ents = [e for e in range(len(T)) if T[e] > 0]
    c = {e: 1 for e in ents}
    while sum(c.values()) < NCORES:
        e = max(ents, key=lambda x: T[x] / c[x])
        c[e] += 1
    assign = []
    for e in ents:
        base, rem = divmod(T[e], c[e])
        t = 0
        for j in range(c[e]):
            nt = base + (1 if j < rem else 0)
            assign.append((e, t, nt))
            t += nt
    assert len(assign) == NCORES
    return assign


def _build(cfg):
    T, n, n0, assign = cfg
    NE = len(T)
    TEM = max(T)
    NQT = max(a[2] for a in assign)
    Ttot = sum(T)
    Stot = Ttot * 128
    SK, SQ = TEM * 128, NQT * 128
    T0 = [min((x + 127) // 128, T[e]) for e, x in enumerate(n0)]
    off = np.cumsum([0] + [t * 128 for t in T]).tolist()

    nc = bacc.Bacc("TRN2", target_bir_lowering=False, debug=False,
                   num_devices=NCORES)
    d = {}
    d["hT"] = nc.dram_tensor("hT", [C, Stot], BF16, kind="ExternalInput")
    d["whead"] = nc.dram_tensor("whead", [C, 480], BF16, kind="ExternalInput")
    d["went"] = nc.dram_tensor("went", [C, 4 * C], BF16, kind="ExternalInput")
    d["woh"] = nc.dram_tensor("woh", [D, C], BF16, kind="ExternalInput")
    d["hq"] = nc.dram_tensor("hq", [C, SQ], BF16, kind="ExternalInput")
    d["hk"] = nc.dram_tensor("hk", [C, SK], BF16, kind="ExternalInput")
    d["entc"] = nc.dram_tensor("entc", [1, 1], F32, kind="ExternalInput")
    out_d = nc.dram_tensor("out", [Stot // 8, C], BF16, kind="ExternalOutput")
    red_d = nc.dram_tensor("red", [Stot // 8, C], BF16)
    eout_d = nc.dram_tensor("eout", [SQ, C], BF16, kind="ExternalOutput")
    P_d = [nc.dram_tensor(f"P{e}", [T[e] * 128, C], BF16) if T[e] else None
           for e in range(NE)]

    with tile.TileContext(nc) as tc:
        _body(nc, tc, d, out_d, red_d, eout_d, P_d, T, T0, n, n0, off,
              TEM, NQT)
    nc.compile()
    return nc


def _body(nc, tc, d, out_d, red_d, eout_d, P_d, T, T0, n, n0, off, TEM,
          NQT):
    NE = len(T)
    Ttot = sum(T)
    Stot = Ttot * 128
    SK, SQ = TEM * 128, NQT * 128
    W4 = 4 * C                      # went row width
    ctx = ExitStack()
    with ctx:
        base = ctx.enter_context(tc.tile_pool(name="base", bufs=1))
        hTb = base.tile([128, 5 * Stot], BF16, tag="hTb")
        wh = base.tile([128, 5 * 480], BF16, tag="wh")
        woh_sb = base.tile([D, C], BF16, tag="woh")
        ones_bf = base.tile([128, 1], BF16, tag="ones_bf")
        ones_f = base.tile([1, 128], F32, tag="ones_f")
        ones_b1 = base.tile([1, 128], BF16, tag="ones_b1")
        entc_sb = base.tile([1, 1], F32, tag="entc")
        nc.vector.memset(ones_bf[:], 1.0)
        nc.vector.memset(ones_f[:], 1.0)
        nc.vector.memset(ones_b1[:], 1.0)
        nc.sync.dma_start(entc_sb[:], d["entc"].ap()[:])
        nc.sync.dma_start(woh_sb[:], d["woh"].ap()[:])
        for cc in range(5):
            nc.sync.dma_start(wh[:, cc * 480:(cc + 1) * 480],
                              d["whead"].ap()[cc * 128:(cc + 1) * 128, :])

        # ================= ENT branch (entity-aligned q slice) ==========
        with tc.tile_pool(name="entp", bufs=1) as ep:
            went = ep.tile([128, 5 * W4], BF16, tag="went")
            hqb = ep.tile([128, 5 * SQ], BF16, tag="hqb")
            hkb = ep.tile([128, 5 * SK], BF16, tag="hkb")
            # load order: q-proj operands first so the PE can start early,
            # then k, v, wof, then the big hT tensor
            for cc in range(5):
                nc.sync.dma_start(
                    went[:, cc * W4:cc * W4 + C],
                    d["went"].ap()[cc * 128:(cc + 1) * 128, 0:C])
                nc.sync.dma_start(hqb[:, cc * SQ:(cc + 1) * SQ],
                                  d["hq"].ap()[cc * 128:(cc + 1) * 128, :])
            for cc in range(5):
                nc.sync.dma_start(
                    went[:, cc * W4 + C:cc * W4 + 2 * C],
                    d["went"].ap()[cc * 128:(cc + 1) * 128, C:2 * C])
                nc.sync.dma_start(hkb[:, cc * SK:(cc + 1) * SK],
                                  d["hk"].ap()[cc * 128:(cc + 1) * 128, :])
            for cc in range(5):
                nc.sync.dma_start(
                    went[:, cc * W4 + 2 * C:(cc + 1) * W4],
                    d["went"].ap()[cc * 128:(cc + 1) * 128, 2 * C:W4])
            for cc in range(5):
                nc.sync.dma_start(hTb[:, cc * Stot:(cc + 1) * Stot],
                                  d["hT"].ap()[cc * 128:(cc + 1) * 128, :])

            qeb = ep.tile([128, 5 * SQ], BF16, tag="qeb")
            keb = ep.tile([128, 5 * SK], BF16, tag="keb")
            veb = ep.tile([128, TEM * C], BF16, tag="veb")
            with tc.tile_pool(name="entps", bufs=2, space="PSUM") as eps:
                for dc in range(5):
                    for f0, fw in _chunks(SQ):
                        pp = eps.tile([128, C], F32, tag="pp")
                        for cc in range(5):
                            nc.tensor.matmul(
                                pp[:, 0:fw],
                                went[:, cc * W4 + dc * 128:cc * W4 + (dc + 1) * 128],
                                hqb[:, cc * SQ + f0:cc * SQ + f0 + fw],
                                start=(cc == 0), stop=(cc == 4))
                        nc.vector.tensor_copy(
                            qeb[:, dc * SQ + f0:dc * SQ + f0 + fw], pp[:, 0:fw])
                for dc in range(5):
                    for f0, fw in _chunks(SK):
                        pp = eps.tile([128, C], F32, tag="pp")
                        for cc in range(5):
                            nc.tensor.matmul(
                                pp[:, 0:fw],
                                went[:, cc * W4 + C + dc * 128:cc * W4 + C + (dc + 1) * 128],
                                hkb[:, cc * SK + f0:cc * SK + f0 + fw],
                                start=(cc == 0), stop=(cc == 4))
                        nc.vector.tensor_copy(
                            keb[:, dc * SK + f0:dc * SK + f0 + fw], pp[:, 0:fw])
                for kt in range(TEM):
                    pp = eps.tile([128, C], F32, tag="pp")
                    for o0, w in ((0, 512), (512, 128)):
                        for cc in range(5):
                            nc.tensor.matmul(
                                pp[:, o0:o0 + w],
                                hkb[:, cc * SK + kt * 128:cc * SK + (kt + 1) * 128],
                                went[:, cc * W4 + 2 * C + o0:cc * W4 + 2 * C + o0 + w],
                                start=(cc == 0), stop=(cc == 4))
                    nc.vector.tensor_copy(veb[:, kt * C:(kt + 1) * C], pp[:])

            oTe = ep.tile([128, 5 * SQ], BF16, tag="oTe")
            PTe = ep.tile([128, TEM * 512], BF16, tag="PTe")
            den_s = ep.tile([1, 512], F32, tag="den_s")
            rec_s = ep.tile([1, 512], F32, tag="rec_s")
            for q0, qw in _chunks(SQ):
                with tc.tile_pool(name="entsc", bufs=2, space="PSUM") as scp, \
                     tc.tile_pool(name="entav", bufs=1, space="PSUM") as avp:
                    pave = avp.tile([128, 5 * 512], F32, tag="pave")
                    pden = avp.tile([1, 512], F32, tag="pden")
                    for kt in range(TEM):
                        pse = scp.tile([128, 512], F32, tag="pse")
                        for dc in range(5):
                            nc.tensor.matmul(
                                pse[:, 0:qw],
                                keb[:, dc * SK + kt * 128:dc * SK + (kt + 1) * 128],
                                qeb[:, dc * SQ + q0:dc * SQ + q0 + qw],
                                start=(dc == 0), stop=(dc == 4))
                        nc.scalar.activation(PTe[:, kt * qw:(kt + 1) * qw],
                                             pse[:, 0:qw], EXP)
                    for kt in range(TEM):
                        for dc in range(5):
                            # dc*512: one PSUM bank per concurrent accum group
                            nc.tensor.matmul(
                                pave[:, dc * 512:dc * 512 + qw],
                                veb[:, kt * C + dc * 128:kt * C + (dc + 1) * 128],
                                PTe[:, kt * qw:(kt + 1) * qw],
                                start=(kt == 0), stop=(kt == TEM - 1))
                        nc.tensor.matmul(pden[:, 0:qw], ones_bf[:],
                                         PTe[:, kt * qw:(kt + 1) * qw],
                                         start=(kt == 0), stop=(kt == TEM - 1))
                    nc.vector.tensor_scalar(den_s[0:1, 0:qw], pden[:, 0:qw],
                                            entc_sb[0:1, 0:1], None, op0=ADD)
                    nc.vector.reciprocal_approx_fast(rec_s[0:1, 0:qw],
                                                     den_s[0:1, 0:qw])
                    rec_b = ep.tile([1, 512], BF16, tag="rec_b")
                    nc.vector.tensor_copy(rec_b[0:1, 0:qw], rec_s[0:1, 0:qw])
                    pB = scp.tile([128, 512], F32, tag="pse", name="pB")
                    nc.tensor.matmul(pB[:, 0:qw], ones_b1[0:1, :],
                                     rec_b[0:1, 0:qw], start=True, stop=True)
                    pBs = ep.tile([128, 512], F32, tag="pBs")
                    nc.vector.tensor_copy(pBs[:, 0:qw], pB[:, 0:qw])
                    for dc in range(5):
                        nc.vector.tensor_mul(
                            oTe[:, dc * SQ + q0:dc * SQ + q0 + qw],
                            pave[:, dc * 512:dc * 512 + qw], pBs[:, 0:qw])
            # ent Wo projection -> eout
            eoutb = ep.tile([128, NQT * C], BF16, tag="eoutb")
            with tc.tile_pool(name="entwo", bufs=2, space="PSUM") as ewp:
                for st in range(NQT):
                    pw = ewp.tile([128, C], F32, tag="pwe")
                    for o0, w in ((0, 512), (512, 128)):
                        for cc in range(5):
                            nc.tensor.matmul(
                                pw[:, o0:o0 + w],
                                oTe[:, cc * SQ + st * 128:cc * SQ + (st + 1) * 128],
                                went[:, cc * W4 + 3 * C + o0:cc * W4 + 3 * C + o0 + w],
                                start=(cc == 0), stop=(cc == 4))
                    nc.scalar.activation(eoutb[:, st * C:(st + 1) * C], pw[:],
                                         COPY)
                for st in range(NQT):
                    nc.sync.dma_start(eout_d.ap()[st * 128:(st + 1) * 128, :],
                                      eoutb[:, st * C:(st + 1) * C])

        # ============ orig + out branches (1 head each per core) ========
        main = ctx.enter_context(tc.tile_pool(name="main", bufs=1))
        qTo = main.tile([D, Stot], BF16, tag="qTo")
        kTo = main.tile([D, Stot], BF16, tag="kTo")
        qTu = main.tile([D, Stot], BF16, tag="qTu")
        kTu = main.tile([D, Stot], BF16, tag="kTu")
        vso = main.tile([128, Ttot * 97], BF16, tag="vso")
        vsu = main.tile([128, Ttot * 97], BF16, tag="vsu")
        hsTo = main.tile([D, Stot], BF16, tag="hsTo")
        hsTu = main.tile([D, Stot], BF16, tag="hsTu")
        hsTs = main.tile([D, Stot], BF16, tag="hsTs")
        PTs = [main.tile([128, TEM * 512], BF16, tag="PT0", name="PT0"),
               main.tile([128, TEM * 512], BF16, tag="PT1", name="PT1")]
        nc.gpsimd.memset(vso[:], 1.0)
        nc.gpsimd.memset(vsu[:], 1.0)

        with tc.tile_pool(name="pjps", bufs=2, space="PSUM") as pjp:
            for dst, wcol, scl in ((qTo, 0, True), (kTo, 80, False),
                                   (qTu, 160, True), (kTu, 240, False)):
                for f0, fw in _chunks(Stot):
                    pq = pjp.tile([D, 512], F32, tag="pq")
                    for cc in range(5):
                        nc.tensor.matmul(
                            pq[:, 0:fw],
                            wh[:, cc * 480 + wcol:cc * 480 + wcol + D],
                            hTb[:, cc * Stot + f0:cc * Stot + f0 + fw],
                            start=(cc == 0), stop=(cc == 4))
                    nc.vector.tensor_copy(dst[:, f0:f0 + fw], pq[:, 0:fw])
            # out-branch boundary tiles: keys n0[e]..T0[e]*128 are im==1 and
            # must not contribute -> zero their k columns and v rows
            bnd = {}
            for e in range(NE):
                if T[e] == 0 or n0[e] == 0 or n0[e] % 128 == 0:
                    continue
                bnd[off[e] // 128 + T0[e] - 1] = n0[e] % 128
                nc.vector.memset(kTu[:, off[e] + n0[e]:off[e] + T0[e] * 128],
                                 0.0)
            for kt in range(Ttot):
                pv = pjp.tile([128, 160], F32, tag="pv")
                for cc in range(5):
                    nc.tensor.matmul(
                        pv[:],
                        hTb[:, cc * Stot + kt * 128:cc * Stot + (kt + 1) * 128],
                        wh[:, cc * 480 + 320:cc * 480 + 480],
                        start=(cc == 0), stop=(cc == 4))
                nc.vector.tensor_copy(vso[:, kt * 97:kt * 97 + 80], pv[:, 0:80])
                if kt in bnd:
                    nc.vector.memset(vsu[:, kt * 97:kt * 97 + 80], 0.0)
                    nc.vector.tensor_copy(vsu[0:bnd[kt], kt * 97:kt * 97 + 80],
                                          pv[0:bnd[kt], 80:160])
                else:
                    nc.vector.tensor_copy(vsu[:, kt * 97:kt * 97 + 80],
                                          pv[:, 80:160])

        # attention + Wo partials + per-entity ReduceScatter
        atx = ExitStack()
        with atx:
            psp = atx.enter_context(tc.tile_pool(name="psp", bufs=2, space="PSUM"))
            avp = atx.enter_context(tc.tile_pool(name="avp", bufs=2, space="PSUM"))
            wop = atx.enter_context(tc.tile_pool(name="wop", bufs=1, space="PSUM"))
            sb2 = atx.enter_context(tc.tile_pool(name="sb2", bufs=2))
            # deep pool so Wo evictions never wait for P-write DMAs that
            # are queued behind a running collective
            pbp = atx.enter_context(tc.tile_pool(name="pbp", bufs=10))
            eorder = sorted([e for e in range(NE) if T[e] > 0],
                            key=lambda e: (-T[e], e))
            # flat job list: (e, branch params, chunk) with a marker on the
            # last chunk of each entity
            jobs = []
            for e in eorder:
                for br, qT, kT, vs, hsT, nkt, corr in (
                        ("o", qTo, kTo, vso, hsTo, T[e],
                         float(n[e] - T[e] * 128)),
                        ("u", qTu, kTu, vsu, hsTu, T0[e],
                         float(n0[e] - T0[e] * 128))):
                    for q0, qw in _chunks(T[e] * 128):
                        jobs.append([e, br, qT, kT, vs, hsT, nkt, corr,
                                     q0, qw, False])
                jobs[-1][10] = True  # entity boundary

            def emit_scores(job, PT):
                e, br, qT, kT, vs, hsT, nkt, corr, q0, qw, last = job
                oe = off[e]
                for g0 in range(0, nkt, 2):
                    gn = min(2, nkt - g0)
                    ps = psp.tile([128, 1024], F32, tag="ps")
                    for j in range(gn):
                        kt = g0 + j
                        nc.tensor.matmul(
                            ps[:, j * qw:j * qw + qw],
                            kT[:, oe + kt * 128:oe + (kt + 1) * 128],
                            qT[:, oe + q0:oe + q0 + qw],
                            start=True, stop=True)
                    nc.scalar.activation(PT[:, g0 * qw:(g0 + gn) * qw],
                                         ps[:, 0:gn * qw], EXP)

            def emit_av(job, PT):
                e, br, qT, kT, vs, hsT, nkt, corr, q0, qw, last = job
                oe = off[e]
                pav = avp.tile([128, 512], F32, tag="pav")
                for kt in range(nkt):
                    nc.tensor.matmul(
                        pav[0:97, 0:qw],
                        vs[:, (oe // 128 + kt) * 97:(oe // 128 + kt) * 97 + 97],
                        PT[:, kt * qw:(kt + 1) * qw],
                        start=(kt == 0), stop=(kt == nkt - 1))
                dn = sb2.tile([1, 512], F32, tag="dn")
                rc = sb2.tile([1, 512], F32, tag="rc")
                rcb = sb2.tile([1, 512], BF16, tag="rcb")
                nc.vector.tensor_scalar(dn[0:1, 0:qw], pav[96:97, 0:qw],
                                        corr, None, op0=ADD)
                nc.vector.reciprocal_approx_fast(rc[0:1, 0:qw], dn[0:1, 0:qw])
                nc.vector.tensor_copy(rcb[0:1, 0:qw], rc[0:1, 0:qw])
                pB = psp.tile([128, 1024], F32, tag="ps", name="pBm")
                nc.tensor.matmul(pB[0:D, 0:qw], ones_b1[0:1, 0:D],
                                 rcb[0:1, 0:qw], start=True, stop=True)
                pBs = sb2.tile([D, 512], F32, tag="pBs")
                nc.vector.tensor_copy(pBs[:, 0:qw], pB[0:D, 0:qw])
                nc.vector.tensor_mul(hsT[:, oe + q0:oe + q0 + qw],
                                     pav[0:D, 0:qw], pBs[:, 0:qw])
                if br == "u":
                    nc.vector.tensor_add(hsTs[:, oe + q0:oe + q0 + qw],
                                         hsTo[:, oe + q0:oe + q0 + qw],
                                         hsTu[:, oe + q0:oe + q0 + qw])

            outoffs = {}
            outoff = 0
            for e in eorder:
                outoffs[e] = outoff
                outoff += T[e] * 16

            def emit_wo_rs(e):
                oe = off[e]
                for st in range(T[e]):
                    gt = oe // 128 + st
                    pw = wop.tile([128, C], F32, tag="pw")
                    for o0, w in ((0, 512), (512, 128)):
                        nc.tensor.matmul(pw[:, o0:o0 + w],
                                         hsTs[:, gt * 128:(gt + 1) * 128],
                                         woh_sb[:, o0:o0 + w],
                                         start=True, stop=True)
                    pbuf = pbp.tile([128, C], BF16, tag="pbuf")
                    nc.scalar.activation(pbuf[:], pw[:], COPY)
                    nc.sync.dma_start(P_d[e].ap()[st * 128:(st + 1) * 128, :],
                                      pbuf[:])
                oo = outoffs[e]
                ne8 = T[e] * 16
                nc.gpsimd.collective_compute(
                    "ReduceScatter", ADD,
                    replica_groups=[list(range(NCORES))],
                    ins=[P_d[e].ap()[:]],
                    outs=[red_d.ap()[oo:oo + ne8, :]])

            # software pipeline: scores(i+1) before av(i) so the exp latency
            # hides behind PE work; Wo/RS fire at entity boundaries
            for i, job in enumerate(jobs):
                emit_scores(job, PTs[i % 2])
                if i > 0:
                    emit_av(jobs[i - 1], PTs[(i - 1) % 2])
                    if jobs[i - 1][10]:
                        emit_wo_rs(jobs[i - 1][0])
            emit_av(jobs[-1], PTs[(len(jobs) - 1) % 2])
            emit_wo_rs(jobs[-1][0])
            # bounce the reduced result through SBUF into the IO tensor
            for r0 in range(0, Stot // 8, 128):
                rw = min(128, Stot // 8 - r0)
                rb = sb2.tile([128, C], BF16, tag="rb")
                nc.sync.dma_start(rb[0:rw, :], red_d.ap()[r0:r0 + rw, :])
                nc.sync.dma_start(out_d.ap()[r0:r0 + rw, :], rb[0:rw, :])


def _plan(mask, inpainting_mask):
    m = np.asarray(mask[0, 0], np.int64)[::8, ::8].reshape(-1)
    im = np.asarray(inpainting_mask[0, 0], np.int64)[::8, ::8].reshape(-1)
    NE = int(m.max()) + 1
    n = [int((m == e).sum()) for e in range(NE)]
    n0 = [int(((m == e) & (im == 0)).sum()) for e in range(NE)]
    for e in range(NE):
        assert n[e] == 0 or n0[e] > 0, "empty outside-key block unsupported"
    T = [(x + 127) // 128 for x in n]
    order = np.lexsort((im, m))
    off = np.cumsum([0] + [t * 128 for t in T])
    pos = np.concatenate([off[e] + np.arange(n[e]) for e in range(NE)
                          if n[e] > 0]).astype(np.int64)
    assign = tuple(_assign_cores(T))
    cfg = (tuple(T), tuple(n), tuple(n0), assign)
    return cfg, order, pos, off


def build_in_maps(hidden_states, mask, inpainting_mask, Wq, Wk, Wv,
                  Wq_ent, Wk_ent, Wv_ent, Wq_out, Wk_out, Wv_out, Wo):
    cfg, order, pos, off = _plan(mask, inpainting_mask)
    T, n, n0, assign = cfg
    TEM = max(T)
    NQT = max(a[2] for a in assign)
    Stot = sum(T) * 128
    SK, SQ = TEM * 128, NQT * 128

    h = np.asarray(hidden_states[0], np.float32)
    hp = np.zeros((Stot, C), np.float32)
    hp[pos] = h[order]
    hTb = np.ascontiguousarray(hp.T).astype(BF)

    def t(W):
        return np.asarray(W, np.float32).T

    went = np.ascontiguousarray(np.concatenate(
        [t(Wq_ent) * SCALE_E, t(Wk_ent), t(Wv_ent), t(Wo)], axis=1)).astype(BF)
    WoT = t(Wo)

    in_maps = []
    for i in range(NCORES):
        hd = slice(D * i, D * (i + 1))
        whead = np.ascontiguousarray(np.concatenate(
            [t(Wq)[:, hd] * SCALE_H, t(Wk)[:, hd],
             t(Wq_out)[:, hd] * SCALE_H, t(Wk_out)[:, hd],
             t(Wv)[:, hd], t(Wv_out)[:, hd]], axis=1)).astype(BF)
        e, t0, nt = assign[i]
        hq = np.zeros((C, SQ), BF)
        hq[:, :nt * 128] = hTb[:, off[e] + t0 * 128:off[e] + (t0 + nt) * 128]
        hk = np.zeros((C, SK), BF)
        hk[:, :T[e] * 128] = hTb[:, off[e]:off[e] + T[e] * 128]
        in_maps.append({
            "hT": hTb, "whead": whead, "went": went,
            "woh": np.ascontiguousarray(WoT[hd, :]).astype(BF),
            "hq": hq, "hk": hk,
            "entc": np.array([[n[e] - TEM * 128]], np.float32),
        })
    _cache["plan"] = (cfg, order, pos, off, hp)
    return in_maps


def kernel(**inputs):
    in_maps = build_in_maps(**inputs)
    cfg, order, pos, off, hp = _cache["plan"]
    T, n, n0, assign = cfg
    Stot = sum(T) * 128
    key = ("nc", cfg)
    if key not in _cache:
        _cache["nc"] = _build(cfg)
        _cache[key] = _cache["nc"]
    res = run_bass_kernel_spmd(_cache[key], in_maps, list(range(NCORES)),
                               trace=False)
    acc = np.zeros((Stot, C), np.float32)
    outoff = 0
    eorder = sorted([e for e in range(len(T)) if T[e] > 0],
                    key=lambda e: (-T[e], e))
    for e in eorder:
        ne8 = T[e] * 16
        for i in range(NCORES):
            acc[off[e] + i * ne8:off[e] + (i + 1) * ne8] = \
                np.asarray(res.results[i]["out"][outoff:outoff + ne8],
                           np.float32)
        outoff += ne8
    for i, (e, t0, nt) in enumerate(assign):
        q0 = off[e] + t0 * 128
        acc[q0:q0 + nt * 128] += np.asarray(
            res.results[i]["eout"][:nt * 128], np.float32)
    acc += hp
    out = np.empty((S, C), np.float32)
    out[order] = acc[pos]
    return out.reshape(1, S, C)


# revision 24
# speedup vs baseline: 3.8544x; 1.0062x over previous
"""Trainium2 Bass kernel for InpaintingAttnProcessor (3-branch masked SDPA).

Block-sparse formulation: the attention masks depend only on 4 entity
labels, so after sorting tokens by (label, inpainting_bit) on the host,
all three SDPA branches become block-diagonal (the "outside" branch
additionally restricts keys to the im==0 prefix of each block).  Each
core computes one head of the two 8-head branches over all blocks, plus
an entity-aligned slice of the single-head d=640 branch.  Per-entity
bf16 ReduceScatters of the Wo partial products overlap the remaining
compute; the entity branch and the residual are assembled on the host.
"""
import numpy as np
import ml_dtypes
from contextlib import ExitStack

import concourse.bass as bass
import concourse.tile as tile
from concourse import bacc, mybir
from concourse.bass_utils import run_bass_kernel_spmd

S, C, H, D = 4096, 640, 8, 80
NCORES = 8
SCALE_H = 1.0 / np.sqrt(80.0)
SCALE_E = 1.0 / np.sqrt(640.0)
F32 = mybir.dt.float32
BF16 = mybir.dt.bfloat16
BF = ml_dtypes.bfloat16
EXP = mybir.ActivationFunctionType.Exp
COPY = mybir.ActivationFunctionType.Copy
ADD = mybir.AluOpType.add

_cache = {}


def _chunks(total, step=512):
    return [(f0, min(step, total - f0)) for f0 in range(0, total, step)]


def _assign_cores(T):
    """Split entity tiles into NCORES contiguous runs, each within one
    entity. Returns list of (entity, tile0_within_entity, ntiles)."""
    ents = [e for e in range(len(T)) if T[e] > 0]
    c = {e: 1 for e in ents}
    while sum(c.values()) < NCORES:
        e = max(ents, key=lambda x: T[x] / c[x])
        c[e] += 1
    assign = []
    for e in ents:
        base, rem = divmod(T[e], c[e])
        t = 0
        for j in range(c[e]):
            nt = base + (1 if j < rem else 0)
            assign.append((e, t, nt))
            t += nt
    assert len(assign) == NCORES
    return assign


def _build(cfg):
    T, n, n0, assign = cfg
    NE = len(T)
    TEM = max(T)
    NQT = max(a[2] for a in assign)
    Ttot = sum(T)
    Stot = Ttot * 128
    SK, SQ = TEM * 128, NQT * 128
    T0 = [min((x + 127) // 128, T[e]) for e, x in enumerate(n0)]
    off = np.cumsum([0] + [t * 128 for t in T]).tolist()

    nc = bacc.Bacc("TRN2", target_bir_lowering=False, debug=False,
                   num_devices=NCORES)
    d = {}
    d["hT"] = nc.dram_tensor("hT", [C, Stot], BF16, kind="ExternalInput")
    d["whead"] = nc.dram_tensor("whead", [C, 480], BF16, kind="ExternalInput")
    d["went"] = nc.dram_tensor("went", [C, 4 * C], BF16, kind="ExternalInput")
    d["woh"] = nc.dram_tensor("woh", [D, C], BF16, kind="ExternalInput")
    d["hq"] = nc.dram_tensor("hq", [C, SQ], BF16, kind="ExternalInput")
    d["hk"] = nc.dram_tensor("hk", [C, SK], BF16, kind="ExternalInput")
    d["entc"] = nc.dram_tensor("entc", [1, 1], F32, kind="ExternalInput")
    eout_d = nc.dram_tensor("eout", [SQ, C], BF16, kind="ExternalOutput")
    P_d = [nc.dram_tensor(f"P{e}", [128, T[e] * C], BF16) if T[e] else None
           for e in range(NE)]
    red_d = [nc.dram_tensor(f"red{e}", [16, T[e] * C], BF16) if T[e] else None
             for e in range(NE)]
    out_d = [nc.dram_tensor(f"o{e}", [16, T[e] * C], BF16,
                            kind="ExternalOutput") if T[e] else None
             for e in range(NE)]

    with tile.TileContext(nc) as tc:
        _body(nc, tc, d, out_d, red_d, eout_d, P_d, T, T0, n, n0, off,
              TEM, NQT)
    nc.compile()
    return nc


def _body(nc, tc, d, out_d, red_d, eout_d, P_d, T, T0, n, n0, off, TEM,
          NQT):
    NE = len(T)
    Ttot = sum(T)
    Stot = Ttot * 128
    SK, SQ = TEM * 128, NQT * 128
    W4 = 4 * C                      # went row width
    ctx = ExitStack()
    with ctx:
        base = ctx.enter_context(tc.tile_pool(name="base", bufs=1))
        hTb = base.tile([128, 5 * Stot], BF16, tag="hTb")
        wh = base.tile([128, 5 * 480], BF16, tag="wh")
        woh_sb = base.tile([D, C], BF16, tag="woh")
        ones_bf = base.tile([128, 1], BF16, tag="ones_bf")
        ones_f = base.tile([1, 128], F32, tag="ones_f")
        ones_b1 = base.tile([1, 128], BF16, tag="ones_b1")
        entc_sb = base.tile([1, 1], F32, tag="entc")
        nc.vector.memset(ones_bf[:], 1.0)
        nc.vector.memset(ones_f[:], 1.0)
        nc.vector.memset(ones_b1[:], 1.0)
        nc.sync.dma_start(entc_sb[:], d["entc"].ap()[:])
        nc.sync.dma_start(woh_sb[:], d["woh"].ap()[:])
        for cc in range(5):
            nc.sync.dma_start(wh[:, cc * 480:(cc + 1) * 480],
                              d["whead"].ap()[cc * 128:(cc + 1) * 128, :])

        # ================= ENT branch (entity-aligned q slice) ==========
        with tc.tile_pool(name="entp", bufs=1) as ep:
            went = ep.tile([128, 5 * W4], BF16, tag="went")
            hqb = ep.tile([128, 5 * SQ], BF16, tag="hqb")
            hkb = ep.tile([128, 5 * SK], BF16, tag="hkb")
            # load order: q-proj operands first so the PE can start early,
            # then k, v, wof, then the big hT tensor
            for cc in range(5):
                nc.sync.dma_start(
                    went[:, cc * W4:cc * W4 + C],
                    d["went"].ap()[cc * 128:(cc + 1) * 128, 0:C])
                nc.sync.dma_start(hqb[:, cc * SQ:(cc + 1) * SQ],
                                  d["hq"].ap()[cc * 128:(cc + 1) * 128, :])
            for cc in range(5):
                nc.sync.dma_start(
                    went[:, cc * W4 + C:cc * W4 + 2 * C],
                    d["went"].ap()[cc * 128:(cc + 1) * 128, C:2 * C])
                nc.sync.dma_start(hkb[:, cc * SK:(cc + 1) * SK],
                                  d["hk"].ap()[cc * 128:(cc + 1) * 128, :])
            for cc in range(5):
                nc.sync.dma_start(
                    went[:, cc * W4 + 2 * C:(cc + 1) * W4],
                    d["went"].ap()[cc * 128:(cc + 1) * 128, 2 * C:W4])
            for cc in range(5):
                nc.sync.dma_start(hTb[:, cc * Stot:(cc + 1) * Stot],
                                  d["hT"].ap()[cc * 128:(cc + 1) * 128, :])

            qeb = ep.tile([128, 5 * SQ], BF16, tag="qeb")
            keb = ep.tile([128, 5 * SK], BF16, tag="keb")
            veb = ep.tile([128, TEM * C], BF16, tag="veb")
            with tc.tile_pool(name="entps", bufs=2, space="PSUM") as eps:
                for dc in range(5):
                    for f0, fw in _chunks(SQ):
                        pp = eps.tile([128, C], F32, tag="pp")
                        for cc in range(5):
                            nc.tensor.matmul(
                                pp[:, 0:fw],
                                went[:, cc * W4 + dc * 128:cc * W4 + (dc + 1) * 128],
                                hqb[:, cc * SQ + f0:cc * SQ + f0 + fw],
                                start=(cc == 0), stop=(cc == 4))
                        nc.vector.tensor_copy(
                            qeb[:, dc * SQ + f0:dc * SQ + f0 + fw], pp[:, 0:fw])
                for dc in range(5):
                    for f0, fw in _chunks(SK):
                        pp = eps.tile([128, C], F32, tag="pp")
                        for cc in range(5):
                            nc.tensor.matmul(
                                pp[:, 0:fw],
                                went[:, cc * W4 + C + dc * 128:cc * W4 + C + (dc + 1) * 128],
                                hkb[:, cc * SK + f0:cc * SK + f0 + fw],
                                start=(cc == 0), stop=(cc == 4))
                        nc.vector.tensor_copy(
                            keb[:, dc * SK + f0:dc * SK + f0 + fw], pp[:, 0:fw])
                for kt in range(TEM):
                    pp = eps.tile([128, C], F32, tag="pp")
                    for o0, w in ((0, 512), (512, 128)):
                        for cc in range(5):
                            nc.tensor.matmul(
                                pp[:, o0:o0 + w],
                                hkb[:, cc * SK + kt * 128:cc * SK + (kt + 1) * 128],
                                went[:, cc * W4 + 2 * C + o0:cc * W4 + 2 * C + o0 + w],
                                start=(cc == 0), stop=(cc == 4))
                    nc.vector.tensor_copy(veb[:, kt * C:(kt + 1) * C], pp[:])

            oTe = ep.tile([128, 5 * SQ], BF16, tag="oTe")
            PTe = ep.tile([128, TEM * 512], BF16, tag="PTe")
            den_s = ep.tile([1, 512], F32, tag="den_s")
            rec_s = ep.tile([1, 512], F32, tag="rec_s")
            for q0, qw in _chunks(SQ):
                with tc.tile_pool(name="entsc", bufs=2, space="PSUM") as scp, \
                     tc.tile_pool(name="entav", bufs=1, space="PSUM") as avp:
                    pave = avp.tile([128, 5 * 512], F32, tag="pave")
                    pden = avp.tile([1, 512], F32, tag="pden")
                    for kt in range(TEM):
                        pse = scp.tile([128, 512], F32, tag="pse")
                        for dc in range(5):
                            nc.tensor.matmul(
                                pse[:, 0:qw],
                                keb[:, dc * SK + kt * 128:dc * SK + (kt + 1) * 128],
                                qeb[:, dc * SQ + q0:dc * SQ + q0 + qw],
                                start=(dc == 0), stop=(dc == 4))
                        nc.scalar.activation(PTe[:, kt * qw:(kt + 1) * qw],
                                             pse[:, 0:qw], EXP)
                    for kt in range(TEM):
                        for dc in range(5):
                            # dc*512: one PSUM bank per concurrent accum group
                            nc.tensor.matmul(
                                pave[:, dc * 512:dc * 512 + qw],
                                veb[:, kt * C + dc * 128:kt * C + (dc + 1) * 128],
                                PTe[:, kt * qw:(kt + 1) * qw],
                                start=(kt == 0), stop=(kt == TEM - 1))
                        nc.tensor.matmul(pden[:, 0:qw], ones_bf[:],
                                         PTe[:, kt * qw:(kt + 1) * qw],
                                         start=(kt == 0), stop=(kt == TEM - 1))
                    nc.vector.tensor_scalar(den_s[0:1, 0:qw], pden[:, 0:qw],
                                            entc_sb[0:1, 0:1], None, op0=ADD)
                    nc.vector.reciprocal_approx_fast(rec_s[0:1, 0:qw],
                                                     den_s[0:1, 0:qw])
                    rec_b = ep.tile([1, 512], BF16, tag="rec_b")
                    nc.vector.tensor_copy(rec_b[0:1, 0:qw], rec_s[0:1, 0:qw])
                    pB = scp.tile([128, 512], F32, tag="pse", name="pB")
                    nc.tensor.matmul(pB[:, 0:qw], ones_b1[0:1, :],
                                     rec_b[0:1, 0:qw], start=True, stop=True)
                    pBs = ep.tile([128, 512], F32, tag="pBs")
                    nc.vector.tensor_copy(pBs[:, 0:qw], pB[:, 0:qw])
                    for dc in range(5):
                        nc.vector.tensor_mul(
                            oTe[:, dc * SQ + q0:dc * SQ + q0 + qw],
                            pave[:, dc * 512:dc * 512 + qw], pBs[:, 0:qw])
            # ent Wo projection -> eout
            eoutb = ep.tile([128, NQT * C], BF16, tag="eoutb")
            with tc.tile_pool(name="entwo", bufs=2, space="PSUM") as ewp:
                for st in range(NQT):
                    pw = ewp.tile([128, C], F32, tag="pwe")
                    for o0, w in ((0, 512), (512, 128)):
                        for cc in range(5):
                            nc.tensor.matmul(
                                pw[:, o0:o0 + w],
                                oTe[:, cc * SQ + st * 128:cc * SQ + (st + 1) * 128],
                                went[:, cc * W4 + 3 * C + o0:cc * W4 + 3 * C + o0 + w],
                                start=(cc == 0), stop=(cc == 4))
                    nc.scalar.activation(eoutb[:, st * C:(st + 1) * C], pw[:],
                                         COPY)
                for st in range(NQT):
                    nc.sync.dma_start(eout_d.ap()[st * 128:(st + 1) * 128, :],
                                      eoutb[:, st * C:(st + 1) * C])

        # ============ orig + out branches (1 head each per core) ========
        main = ctx.enter_context(tc.tile_pool(name="main", bufs=1))
        qTo = main.tile([D, Stot], BF16, tag="qTo")
        kTo = main.tile([D, Stot], BF16, tag="kTo")
        qTu = main.tile([D, Stot], BF16, tag="qTu")
        kTu = main.tile([D, Stot], BF16, tag="kTu")
        vso = main.tile([128, Ttot * 97], BF16, tag="vso")
        vsu = main.tile([128, Ttot * 97], BF16, tag="vsu")
        hsTo = main.tile([D, Stot], BF16, tag="hsTo")
        hsTu = main.tile([D, Stot], BF16, tag="hsTu")
        hsTs = main.tile([D, Stot], BF16, tag="hsTs")
        PTs = [main.tile([128, TEM * 512], BF16, tag="PT0", name="PT0"),
               main.tile([128, TEM * 512], BF16, tag="PT1", name="PT1")]
        nc.gpsimd.memset(vso[:], 1.0)
        nc.gpsimd.memset(vsu[:], 1.0)

        with tc.tile_pool(name="pjps", bufs=2, space="PSUM") as pjp:
            for dst, wcol, scl in ((qTo, 0, True), (kTo, 80, False),
                                   (qTu, 160, True), (kTu, 240, False)):
                for f0, fw in _chunks(Stot):
                    pq = pjp.tile([D, 512], F32, tag="pq")
                    for cc in range(5):
                        nc.tensor.matmul(
                            pq[:, 0:fw],
                            wh[:, cc * 480 + wcol:cc * 480 + wcol + D],
                            hTb[:, cc * Stot + f0:cc * Stot + f0 + fw],
                            start=(cc == 0), stop=(cc == 4))
                    nc.vector.tensor_copy(dst[:, f0:f0 + fw], pq[:, 0:fw])
            # out-branch boundary tiles: keys n0[e]..T0[e]*128 are im==1 and
            # must not contribute -> zero their k columns and v rows
            bnd = {}
            for e in range(NE):
                if T[e] == 0 or n0[e] == 0 or n0[e] % 128 == 0:
                    continue
                bnd[off[e] // 128 + T0[e] - 1] = n0[e] % 128
                nc.vector.memset(kTu[:, off[e] + n0[e]:off[e] + T0[e] * 128],
                                 0.0)
            for kt in range(Ttot):
                pv = pjp.tile([128, 160], F32, tag="pv")
                for cc in range(5):
                    nc.tensor.matmul(
                        pv[:],
                        hTb[:, cc * Stot + kt * 128:cc * Stot + (kt + 1) * 128],
                        wh[:, cc * 480 + 320:cc * 480 + 480],
                        start=(cc == 0), stop=(cc == 4))
                nc.vector.tensor_copy(vso[:, kt * 97:kt * 97 + 80], pv[:, 0:80])
                if kt in bnd:
                    nc.vector.memset(vsu[:, kt * 97:kt * 97 + 80], 0.0)
                    nc.vector.tensor_copy(vsu[0:bnd[kt], kt * 97:kt * 97 + 80],
                                          pv[0:bnd[kt], 80:160])
                else:
                    nc.vector.tensor_copy(vsu[:, kt * 97:kt * 97 + 80],
                                          pv[:, 80:160])

        # attention + Wo partials + per-entity ReduceScatter
        atx = ExitStack()
        with atx:
            psp = atx.enter_context(tc.tile_pool(name="psp", bufs=2, space="PSUM"))
            avp = atx.enter_context(tc.tile_pool(name="avp", bufs=2, space="PSUM"))
            wop = atx.enter_context(tc.tile_pool(name="wop", bufs=1, space="PSUM"))
            sb2 = atx.enter_context(tc.tile_pool(name="sb2", bufs=2))
            # deep pool so Wo evictions never wait for P-write DMAs that
            # are queued behind a running collective
            pbp = atx.enter_context(tc.tile_pool(name="pbp", bufs=2))
            eorder = sorted([e for e in range(NE) if T[e] > 0],
                            key=lambda e: (-T[e], e))
            # flat job list: (e, branch params, chunk) with a marker on the
            # last chunk of each entity
            jobs = []
            for e in eorder:
                for br, qT, kT, vs, hsT, nkt, corr in (
                        ("o", qTo, kTo, vso, hsTo, T[e],
                         float(n[e] - T[e] * 128)),
                        ("u", qTu, kTu, vsu, hsTu, T0[e],
                         float(n0[e] - T0[e] * 128))):
                    for q0, qw in _chunks(T[e] * 128):
                        jobs.append([e, br, qT, kT, vs, hsT, nkt, corr,
                                     q0, qw, False])
                jobs[-1][10] = True  # entity boundary

            def emit_scores(job, PT):
                e, br, qT, kT, vs, hsT, nkt, corr, q0, qw, last = job
                oe = off[e]
                for g0 in range(0, nkt, 2):
                    gn = min(2, nkt - g0)
                    ps = psp.tile([128, 1024], F32, tag="ps")
                    for j in range(gn):
                        kt = g0 + j
                        nc.tensor.matmul(
                            ps[:, j * qw:j * qw + qw],
                            kT[:, oe + kt * 128:oe + (kt + 1) * 128],
                            qT[:, oe + q0:oe + q0 + qw],
                            start=True, stop=True)
                    nc.scalar.activation(PT[:, g0 * qw:(g0 + gn) * qw],
                                         ps[:, 0:gn * qw], EXP)

            def emit_av(job, PT):
                e, br, qT, kT, vs, hsT, nkt, corr, q0, qw, last = job
                oe = off[e]
                pav = avp.tile([128, 512], F32, tag="pav")
                for kt in range(nkt):
                    nc.tensor.matmul(
                        pav[0:97, 0:qw],
                        vs[:, (oe // 128 + kt) * 97:(oe // 128 + kt) * 97 + 97],
                        PT[:, kt * qw:(kt + 1) * qw],
                        start=(kt == 0), stop=(kt == nkt - 1))
                dn = sb2.tile([1, 512], F32, tag="dn")
                rc = sb2.tile([1, 512], F32, tag="rc")
                rcb = sb2.tile([1, 512], BF16, tag="rcb")
                nc.vector.tensor_scalar(dn[0:1, 0:qw], pav[96:97, 0:qw],
                                        corr, None, op0=ADD)
                nc.vector.reciprocal_approx_fast(rc[0:1, 0:qw], dn[0:1, 0:qw])
                nc.vector.tensor_copy(rcb[0:1, 0:qw], rc[0:1, 0:qw])
                pB = psp.tile([128, 1024], F32, tag="ps", name="pBm")
                nc.tensor.matmul(pB[0:D, 0:qw], ones_b1[0:1, 0:D],
                                 rcb[0:1, 0:qw], start=True, stop=True)
                pBs = sb2.tile([D, 512], F32, tag="pBs")
                nc.vector.tensor_copy(pBs[:, 0:qw], pB[0:D, 0:qw])
                nc.vector.tensor_mul(hsT[:, oe + q0:oe + q0 + qw],
                                     pav[0:D, 0:qw], pBs[:, 0:qw])
                if br == "u":
                    nc.vector.tensor_add(hsTs[:, oe + q0:oe + q0 + qw],
                                         hsTo[:, oe + q0:oe + q0 + qw],
                                         hsTu[:, oe + q0:oe + q0 + qw])

            def emit_wo_rs(e):
                oe = off[e]
                pbatch = pbp.tile([128, TEM * C], BF16, tag="pbatch")
                for st in range(T[e]):
                    gt = oe // 128 + st
                    pw = wop.tile([128, C], F32, tag="pw")
                    for o0, w in ((0, 512), (512, 128)):
                        nc.tensor.matmul(pw[:, o0:o0 + w],
                                         hsTs[:, gt * 128:(gt + 1) * 128],
                                         woh_sb[:, o0:o0 + w],
                                         start=True, stop=True)
                    nc.scalar.activation(pbatch[:, st * C:(st + 1) * C],
                                         pw[:], COPY)
                nc.sync.dma_start(P_d[e].ap()[:], pbatch[:, 0:T[e] * C])
                nc.gpsimd.collective_compute(
                    "ReduceScatter", ADD,
                    replica_groups=[list(range(NCORES))],
                    ins=[P_d[e].ap()[:]],
                    outs=[red_d[e].ap()[:]])
                nc.sync.dma_start(out_d[e].ap()[:], red_d[e].ap()[:])

            # software pipeline: scores(i+1) before av(i) so the exp latency
            # hides behind PE work; Wo/RS fire at entity boundaries
            for i, job in enumerate(jobs):
                emit_scores(job, PTs[i % 2])
                if i > 0:
                    emit_av(jobs[i - 1], PTs[(i - 1) % 2])
                    if jobs[i - 1][10]:
                        emit_wo_rs(jobs[i - 1][0])
            emit_av(jobs[-1], PTs[(len(jobs) - 1) % 2])
            emit_wo_rs(jobs[-1][0])


def _plan(mask, inpainting_mask):
    m = np.asarray(mask[0, 0], np.int64)[::8, ::8].reshape(-1)
    im = np.asarray(inpainting_mask[0, 0], np.int64)[::8, ::8].reshape(-1)
    NE = int(m.max()) + 1
    n = [int((m == e).sum()) for e in range(NE)]
    n0 = [int(((m == e) & (im == 0)).sum()) for e in range(NE)]
    for e in range(NE):
        assert n[e] == 0 or n0[e] > 0, "empty outside-key block unsupported"
    T = [(x + 127) // 128 for x in n]
    order = np.lexsort((im, m))
    off = np.cumsum([0] + [t * 128 for t in T])
    pos = np.concatenate([off[e] + np.arange(n[e]) for e in range(NE)
                          if n[e] > 0]).astype(np.int64)
    assign = tuple(_assign_cores(T))
    cfg = (tuple(T), tuple(n), tuple(n0), assign)
    return cfg, order, pos, off


def build_in_maps(hidden_states, mask, inpainting_mask, Wq, Wk, Wv,
                  Wq_ent, Wk_ent, Wv_ent, Wq_out, Wk_out, Wv_out, Wo):
    cfg, order, pos, off = _plan(mask, inpainting_mask)
    T, n, n0, assign = cfg
    TEM = max(T)
    NQT = max(a[2] for a in assign)
    Stot = sum(T) * 128
    SK, SQ = TEM * 128, NQT * 128

    h = np.asarray(hidden_states[0], np.float32)
    hp = np.zeros((Stot, C), np.float32)
    hp[pos] = h[order]
    hTb = np.ascontiguousarray(hp.T).astype(BF)

    def t(W):
        return np.asarray(W, np.float32).T

    went = np.ascontiguousarray(np.concatenate(
        [t(Wq_ent) * SCALE_E, t(Wk_ent), t(Wv_ent), t(Wo)], axis=1)).astype(BF)
    WoT = t(Wo)

    in_maps = []
    for i in range(NCORES):
        hd = slice(D * i, D * (i + 1))
        whead = np.ascontiguousarray(np.concatenate(
            [t(Wq)[:, hd] * SCALE_H, t(Wk)[:, hd],
             t(Wq_out)[:, hd] * SCALE_H, t(Wk_out)[:, hd],
             t(Wv)[:, hd], t(Wv_out)[:, hd]], axis=1)).astype(BF)
        e, t0, nt = assign[i]
        hq = np.zeros((C, SQ), BF)
        hq[:, :nt * 128] = hTb[:, off[e] + t0 * 128:off[e] + (t0 + nt) * 128]
        hk = np.zeros((C, SK), BF)
        hk[:, :T[e] * 128] = hTb[:, off[e]:off[e] + T[e] * 128]
        in_maps.append({
            "hT": hTb, "whead": whead, "went": went,
            "woh": np.ascontiguousarray(WoT[hd, :]).astype(BF),
            "hq": hq, "hk": hk,
            "entc": np.array([[n[e] - TEM * 128]], np.float32),
        })
    _cache["plan"] = (cfg, order, pos, off, hp)
    return in_maps


def kernel(**inputs):
    in_maps = build_in_maps(**inputs)
    cfg, order, pos, off, hp = _cache["plan"]
    T, n, n0, assign = cfg
    Stot = sum(T) * 128
    key = ("nc", cfg)
    if key not in _cache:
        _cache["nc"] = _build(cfg)
        _cache[key] = _cache["nc"]
    res = run_bass_kernel_spmd(_cache[key], in_maps, list(range(NCORES)),
                               trace=False)
    acc = np.zeros((Stot, C), np.float32)
    for e in range(len(T)):
        if T[e] == 0:
            continue
        acc_e = acc[off[e]:off[e] + T[e] * 128].reshape(T[e], 128, C)
        for i in range(NCORES):
            blk = np.asarray(res.results[i][f"o{e}"],
                             np.float32).reshape(16, T[e], C)
            acc_e[:, 16 * i:16 * (i + 1), :] = blk.transpose(1, 0, 2)
    for i, (e, t0, nt) in enumerate(assign):
        q0 = off[e] + t0 * 128
        acc[q0:q0 + nt * 128] += np.asarray(
            res.results[i]["eout"][:nt * 128], np.float32)
    acc += hp
    out = np.empty((S, C), np.float32)
    out[order] = acc[pos]
    return out.reshape(1, S, C)
